# revision 1
# baseline (speedup 1.0000x reference)
"""Trainium2 Bass kernel for a pre-norm transformer block (nn_Block).

Math (per batch b of x [4, 1024, 1024]):
    h  = LN(x) ; qkv = h @ w_qkv + b_qkv ; attention (16 heads, dh=64)
    x  = x + (attn_out @ w_proj + b_proj)
    h  = LN(x) ; x = x + gelu(h @ w_fc1 + b_fc1) @ w_fc2 + b_fc2

Sharding: communication-free hybrid over 8 cores. Core c handles batch
b = c // 2 and query-token half c % 2. Each core computes K and V for its
batch's full 1024 tokens and everything else for its own 512 queries.

Precision split (validated against the fp32 reference in numpy and on HW):
  - K/Q/V production, S^T, exp(P) and PV run in fp8-e4m3, with DoubleRow
    perf mode (2 contraction blocks per matmul) for K/Q/V/PV. Softmax's
    averaging over ~1024 keys washes the quantization noise out
    (end-to-end rel_l2 ~2.0e-3 vs 1.5e-3 all-bf16).
  - proj and the MLP stay bf16: fp8 there pushes rel_l2 past the 2e-2
    gate (measured 2.4e-2), and at DoubleRow's ~1.5x win any hi/lo
    compensation scheme costs more than bf16.

Layout is feature-major ([features, tokens]); fp8 weights are host-packed
into DoubleRow pair layout (contraction c = (2j+i)*128+p). S^T accumulates
key-block pairs into 2-bank [128,1024] psum tiles so one exp covers both,
and the exp's fp8 output tile is directly the PV DoubleRow moving operand.
Softmax denominators ride a ones-column through PV (psum row 64); ACT
computes 1/den = Exp(-Ln(den)) off the psum row, the rows bounce through
DRAM into a [128,512] broadcast, and one DVE multiply normalizes both
heads' O. LN stats come from ones-vector matmuls (squares split across
DVE/ACT), the fused row chain folds 1/C into the Ln scale, and the
per-token scale/shift rows replicate across partitions via a bf16 DRAM
bounce. Scheduling notes: bulk proj/MLP weight loads are dependency-
anchored so their DMA bursts land inside matmul-heavy phases (an SBUF-port
collision with the LN bounce or the h1 normalize triples DVE op latency);
K/Q production for the first two head pairs is interleaved into the V
matmul groups so the attention pipeline starts while V casts drain.

Measured on 8 axon-tunneled trn2 cores: 342.6 us vs the 405 us bf16
baseline; PE busy ~254 us (down from 303) of which the bf16 proj+MLP
matmuls are 171 us at roofline. The S^T-score psum pool is released
before the final PV so proj matmuls overlap the last denominator chain
(a pad pool steers proj psums onto the early-freed banks).
"""

import os
import sys

import numpy as np

try:
    import concourse.bass as bass
except ImportError:  # pragma: no cover
    for _p in ("/opt/trn_rl_repo", "/root/.axon_site/_ro/trn_rl_repo"):
        if os.path.isdir(_p) and _p not in sys.path:
            sys.path.insert(0, _p)
    import concourse.bass as bass

import ml_dtypes
import concourse.tile as tile
import concourse.mybir as mybir
from concourse import bass_utils
from concourse.bass import ds

F32 = mybir.dt.float32
BF16 = mybir.dt.bfloat16
FP8 = mybir.dt.float8e4
AF = mybir.ActivationFunctionType
DR = mybir.MatmulPerfMode.DoubleRow

C = 1024          # model dim
H = 16            # heads
DH = 64           # head dim
NTOK = 1024       # tokens per batch (keys/values)
NQ = 512          # query tokens per core
KT = C // 128     # 8 feature tiles
JT = KT // 2      # 4 feature-pair tiles (DoubleRow)
HID = 4096
MT1 = HID // 128  # 32 fc1 output tiles
EPS = 1e-5
WSCALE = 2048.0   # pow2 scale folded into fp8 qkv weights
QSM = 1.0 / (WSCALE * float(DH) ** 0.5)   # Q psum -> fp8 cast scale
KSM = 1.0 / WSCALE                        # K/V psum -> fp8 cast scale

_cache = {}


def _split_wide_waits(nc, max_waits=1):
    """Walrus on this image rejects instructions carrying more than one
    semaphore wait; split the excess onto same-engine NOPs."""
    ctr = 0
    for f in nc.m.functions:
        for b in f.blocks:
            out, changed = [], False
            for inst in b.instructions:
                si = getattr(inst, "sync_info", None)
                if si is not None and si.on_wait and len(si.on_wait) > max_waits:
                    waits = list(si.on_wait)
                    extra, keep = waits[:-max_waits], waits[-max_waits:]
                    for gs in range(0, len(extra), max_waits):
                        ctr += 1
                        nop = mybir.InstNoOp(
                            name=f"waitsplit-{ctr}", ins=[], outs=[])
                        nop.engine = inst.engine
                        nop.sync_info = mybir.SyncInfo(
                            on_wait=extra[gs:gs + max_waits], on_update=[])
                        out.append(nop)
                    inst.sync_info = mybir.SyncInfo(
                        on_wait=keep, on_update=list(si.on_update))
                    changed = True
                out.append(inst)
            if changed:
                b.instructions = out


def build_program(has_bias, gelu_func=None):
    nc = bass.Bass()

    # packed layouts: >=8KB contiguous per partition line per DMA
    xTp = nc.dram_tensor("xTp", [128, KT * NQ], F32, kind="ExternalInput")
    xbp = nc.dram_tensor("xbp", [128, KT * NTOK], BF16, kind="ExternalInput")
    wkq8 = nc.dram_tensor("wkq8", [KT, 128, 2 * C], FP8, kind="ExternalInput")
    wv8 = nc.dram_tensor("wv8", [128, JT * 2 * C], FP8, kind="ExternalInput")
    wpp = nc.dram_tensor("wpp", [128, KT * C], BF16, kind="ExternalInput")
    w1g = nc.dram_tensor("w1g", [MT1 // 4, 128, 4 * C], BF16,
                         kind="ExternalInput")
    w2_m = nc.dram_tensor("w2_m", [KT, 128, HID], BF16, kind="ExternalInput")
    b_all = nc.dram_tensor("b_all", [1, 3 * C + C + HID + C], BF16,
                           kind="ExternalInput")
    yT = nc.dram_tensor("yT", [C, NQ], F32, kind="ExternalOutput")

    with tile.TileContext(nc) as tc:
        _emit(nc, tc, xTp, xbp, wkq8, wv8, wpp, w1g, w2_m, b_all,
              yT, has_bias, gelu_func or AF.Gelu)
    return nc


def _emit(nc, tc, xTp, xbp, wkq8, wv8, wpp, w1g, w2_m, b_all, yT,
          has_bias, gelu_func):
    pers = tc.alloc_tile_pool(name="pers", bufs=1)
    ones_c = pers.tile([128, 1], BF16, tag="ones_c")      # stats lhsT
    nc.vector.memset(ones_c, 1.0)
    ones_r16 = pers.tile([1, NQ], BF16, tag="ones_r16")   # bias rank-1 rhs
    nc.vector.memset(ones_r16, 1.0)
    ones_tok16 = pers.tile([1, 128], BF16, tag="ones_tok16")  # v-bias lhsT
    nc.vector.memset(ones_tok16, 1.0)
    eps_t = pers.tile([128, 1], F32, tag="eps_t")
    nc.vector.memset(eps_t, EPS)

    p_dram = tc.alloc_tile_pool(name="dscratch", bufs=4, space="DRAM")

    def ln_chain(ms, ss, N, pool, nm, ps_warm=None, wtag=None):
        """From per-chunk sum/sumsq PSUM rows, produce [128, N] bf16
        rstd_rep and (mu*rstd)_rep via a bf16 DRAM bounce. Fused row math:
        1/C folds into the Ln scale, mu*rstd is one scalar_tensor_tensor."""
        nch = N // 512
        out2 = pool.tile([1, 2 * N], BF16, tag=f"out2_{nm}",
                         name=f"out2_{nm}")
        for n in range(nch):
            o = n * 512
            tn = pool.tile([1, 512], F32, tag=f"tn_{nm}", name=f"tn_{nm}{n}")
            nc.scalar.activation(tn, ms[n], AF.Square)
            vn = pool.tile([1, 512], F32, tag=f"vn_{nm}", name=f"vn_{nm}{n}")
            nc.vector.scalar_tensor_tensor(
                vn, tn, 1.0 / C, ss[n],
                mybir.AluOpType.mult, mybir.AluOpType.subtract)
            lnv = pool.tile([1, 512], F32, tag=f"lnv_{nm}",
                            name=f"lnv_{nm}{n}")
            nc.scalar.activation(lnv, vn, AF.Ln, bias=eps_t[ds(0, 1), :],
                                 scale=-1.0 / C)
            nc.scalar.activation(out2[:, ds(o, 512)], lnv, AF.Exp, scale=-0.5)
            rstf = pool.tile([1, 512], F32, tag=f"rstf_{nm}",
                             name=f"rstf_{nm}{n}")
            nc.scalar.activation(rstf, lnv, AF.Exp, scale=-0.5)
            nc.vector.scalar_tensor_tensor(
                out2[:, ds(N + o, 512)], ms[n], 1.0 / C, rstf,
                mybir.AluOpType.mult, mybir.AluOpType.mult)
        drow = p_dram.tile([1, 2 * N], BF16, tag="dscratch", name=f"dr_{nm}")
        nc.gpsimd.dma_start(drow, out2)
        rep = pool.tile([128, 2 * N], BF16, tag=f"rep_{nm}", name=f"rep_{nm}")
        nc.gpsimd.dma_start(rep[ds(0, 64), :], drow.to_broadcast((64, 2 * N)))
        nc.sync.dma_start(rep[ds(64, 64), :], drow.to_broadcast((64, 2 * N)))
        return rep[:, ds(0, N)], rep[:, ds(N, N)]

    any_bias = any(has_bias.values())
    if any_bias:
        bias_sb = pers.tile([1, 3 * C + C + HID + C], BF16, tag="bias_sb")
        nc.sync.dma_start(bias_sb, b_all[:])
        bq_of, bk_of, bv_of = 0, C, 2 * C
        bp_of, b1_of, b2_of = 3 * C, 4 * C, 4 * C + HID

    # pools ordered by lifetime (latest-dying first): releases are a stack
    p_x2 = tc.alloc_tile_pool(name="x2", bufs=KT)
    p_w2 = tc.alloc_tile_pool(name="w2", bufs=3)
    p_w1 = tc.alloc_tile_pool(name="w1", bufs=3)
    p_xT = tc.alloc_tile_pool(name="xT", bufs=1)
    p_wp = tc.alloc_tile_pool(name="wp", bufs=1)
    p_V = tc.alloc_tile_pool(name="V", bufs=JT)
    p_wv = tc.alloc_tile_pool(name="wv", bufs=1)
    p_h1 = tc.alloc_tile_pool(name="h1", bufs=JT)
    p_O = tc.alloc_tile_pool(name="O", bufs=KT)
    p_wkq = tc.alloc_tile_pool(name="wkq", bufs=4)
    p_xb = tc.alloc_tile_pool(name="xb", bufs=1)
    p_ln1 = tc.alloc_tile_pool(name="ln1", bufs=1)
    p_sq = tc.alloc_tile_pool(name="sq", bufs=3)
    ps_stat = tc.alloc_tile_pool(name="ps_stat", bufs=1, space="PSUM")

    # ---- bulk loads: one big DMA each, SBUF views per tile ----
    xb_all = p_xb.tile([128, KT * NTOK], BF16, tag="xb")
    for q in range(4):
        nc.sync.dma_start(xb_all[:, ds(q * 2 * NTOK, 2 * NTOK)],
                          xbp[:, ds(q * 2 * NTOK, 2 * NTOK)])
    xbt = [xb_all[:, ds(k * NTOK, NTOK)] for k in range(KT)]

    wv_all = p_wv.tile([128, JT * 2 * C], FP8, tag="wv")
    nc.sync.dma_start(wv_all, wv8[:, :])
    wv = [wv_all[:, ds(j * 2 * C, 2 * C)] for j in range(JT)]

    # prefetch K/Q weights for the first head pair only (keeps the DMA
    # queues clear for the LN1 row bounce)
    wkq_tiles = {}
    for t0 in range(2):
        w = p_wkq.tile([128, 2 * C], FP8, tag="wkq", name=f"wkq{t0}")
        nc.sync.dma_start(w, wkq8[t0, :, :])
        wkq_tiles[t0] = w

    # V2[r]: pair layout [128 tok, i(2), H, 65] fp8; [.., 64] is the ones col
    V2 = []
    for r in range(JT):
        vt = p_V.tile([128, 2, H, 65], FP8, tag="V", name=f"V{r}")
        nc.vector.memset(vt[:, :, :, ds(64, 1)], 1.0)
        V2.append(vt)

    # ---- LN1 stats; squares rotate across DVE / ACT / GpSimd ----
    ms = [ps_stat.tile([1, 512], F32, tag=f"ms{n}", name=f"ms{n}")
          for n in range(2)]
    ss = [ps_stat.tile([1, 512], F32, tag=f"ss{n}", name=f"ss{n}")
          for n in range(2)]
    for k in range(KT):
        sq = p_sq.tile([128, NTOK], BF16, tag="sq")
        if k % 4 == 2:
            nc.scalar.activation(sq, xbt[k], AF.Square)
        else:
            nc.vector.tensor_mul(sq, xbt[k], xbt[k])
        for n in range(2):
            nc.tensor.matmul(ms[n], ones_c, xbt[k][:, ds(n * 512, 512)],
                             start=(k == 0), stop=(k == KT - 1))
            nc.tensor.matmul(ss[n], ones_c, sq[:, ds(n * 512, 512)],
                             start=(k == 0), stop=(k == KT - 1))
    p_sq.release()

    rstd_rep, musc_rep = ln_chain(ms, ss, NTOK, p_ln1, "ln1",
                                  ps_warm=ps_stat, wtag="w1t")
    ps_stat.release()

    # bulk prefetch for proj/MLP. The tiny seed copies make each DMA
    # depend on the LN1 broadcast, so their descriptors cannot be hoisted
    # ahead of the LN1 bounce; the loads then fill the queues during
    # V/attention when nothing is latency-critical.
    xt = p_xT.tile([128, KT * NQ], F32, tag="xT")
    wp_all = p_wp.tile([128, KT * C], BF16, tag="wp")

    # h1p[j]: fp8 pair tile [128, 2, NTOK]; halves are feature blocks 2j,2j+1
    p_tmp = tc.alloc_tile_pool(name="tmp", bufs=4)
    h1 = [p_h1.tile([128, 2, NTOK], FP8, tag="h1", name=f"h1p{j}")
          for j in range(JT)]
    for k in range(KT):
        eng = nc.gpsimd if k % 3 == 2 else nc.vector
        tmp = p_tmp.tile([128, NTOK], F32, tag="tmp")
        eng.tensor_mul(tmp, xbt[k], rstd_rep)
        eng.tensor_sub(h1[k // 2][:, k % 2, :], tmp, musc_rep)
    p_tmp.release()

    # bulk proj-phase loads: anchored on the last h1 tile so the 6MB burst
    # lands during the matmul-heavy V/attention phases, not during the
    # vector-heavy LN1 window
    nc.vector.tensor_copy(xt[ds(0, 1), ds(0, 1)], h1[3][ds(0, 1), 1, ds(0, 1)])
    nc.sync.dma_start(xt, xTp[:, :])
    nc.vector.tensor_copy(wp_all[ds(0, 1), ds(0, 1)],
                          h1[3][ds(0, 1), 1, ds(0, 1)])
    nc.sync.dma_start(wp_all, wpp[:, :])
    p_ln1.release()
    p_xb.release()

    # ---------------- K/Q psum pool first: reserves 2 banks ----
    ps_kq = tc.alloc_tile_pool(name="ps_kq", bufs=2, space="PSUM")
    ps_v = tc.alloc_tile_pool(name="ps_v", bufs=6, space="PSUM")

    # ---------------- V (token-major, DoubleRow over feature pairs) -----

    # ---------------- attention loop over head pairs --------------------
    p_K = tc.alloc_tile_pool(name="K", bufs=KT)
    p_Q = tc.alloc_tile_pool(name="Q", bufs=KT)
    p_P = tc.alloc_tile_pool(name="P", bufs=18)
    p_rq = tc.alloc_tile_pool(name="rq", bufs=4)
    p_rep = tc.alloc_tile_pool(name="rep", bufs=2)

    K_sb, Q_sb, P_sb, O_sb = [], [], {}, []

    def emit_kq(t):
        wt = wkq_tiles.pop(t)
        wkt, wqt = wt[:, ds(0, C)], wt[:, ds(C, C)]
        if t + 2 < KT:  # keep two pairs in flight
            nw = p_wkq.tile([128, 2 * C], FP8, tag="wkq", name=f"wkq{t+2}")
            nc.sync.dma_start(nw, wkq8[t + 2, :, :])
            wkq_tiles[t + 2] = nw
        kt_sb = p_K.tile([128, NTOK], FP8, tag="K")
        wkp = wkt.rearrange("p (j i f) -> p j i f", j=JT, i=2)
        for n in range(2):
            ps = ps_kq.tile([128, 512], F32, tag="ps_kq")
            for j in range(JT):
                nc.tensor.matmul(
                    ps, wkp[:, j],
                    h1[j][:, :, ds(n * 512, 512)],
                    start=(j == 0), stop=(j == JT - 1 and not has_bias["qk"]),
                    perf_mode=DR)
            if has_bias["qk"]:
                nc.tensor.matmul(
                    ps, bias_sb[:, ds(bk_of + t * 128, 128)], ones_r16,
                    start=False, stop=True)
            nc.vector.tensor_scalar_mul(kt_sb[:, ds(n * 512, 512)], ps, KSM)
        K_sb.append(kt_sb)

        qt_sb = p_Q.tile([128, NQ], FP8, tag="Q")
        wqp = wqt.rearrange("p (j i f) -> p j i f", j=JT, i=2)
        ps = ps_kq.tile([128, 512], F32, tag="ps_kq")
        for j in range(JT):
            nc.tensor.matmul(
                ps, wqp[:, j], h1[j][:, :, ds(0, 512)],
                start=(j == 0), stop=(j == JT - 1 and not has_bias["qk"]),
                perf_mode=DR)
        if has_bias["qk"]:
            nc.tensor.matmul(
                ps, bias_sb[:, ds(bq_of + t * 128, 128)], ones_r16,
                start=False, stop=True)
        nc.vector.tensor_scalar_mul(qt_sb, ps, QSM)
        Q_sb.append(qt_sb)

    kq_early = [lambda: emit_kq(0), lambda: emit_kq(1)]

    for g0 in range(0, KT, 3):
        if g0 > 0 and kq_early:
            kq_early.pop(0)()   # emit_kq(0)/(1) between V groups
        ts_ = range(g0, min(g0 + 3, KT))
        psv = {(t, n): ps_v.tile([128, 512], F32, tag="ps_v",
                                 name=f"psv{t}_{n}")
               for t in ts_ for n in range(2)}
        for j in range(JT):
            for t in ts_:
                for n in range(2):
                    nc.tensor.matmul(
                        psv[(t, n)], h1[j][:, :, ds(t * 128, 128)],
                        wv[j].rearrange("p (i f) -> p i f", i=2)[
                            :, :, ds(n * 512, 512)],
                        start=(j == 0),
                        stop=(j == JT - 1 and not has_bias["v"]),
                        perf_mode=DR)
        for t in ts_:
            for n in range(2):
                if has_bias["v"]:
                    nc.tensor.matmul(
                        psv[(t, n)], ones_tok16,
                        bias_sb[:, ds(bv_of + n * 512, 512)],
                        start=False, stop=True)
                nc.vector.tensor_scalar_mul(
                    V2[t // 2][:, t % 2, ds(n * 8, 8), ds(0, 64)],
                    psv[(t, n)].rearrange("p (h d) -> p h d", d=64), KSM)
    ps_v.release()
    ps_o = tc.alloc_tile_pool(name="ps_o", bufs=2, space="PSUM")
    ps_s = tc.alloc_tile_pool(name="ps_s", bufs=2, space="PSUM")

    def emit_st(t):
        # S^T per key-block pair r: [128,1024] psum (2 banks) per head; the
        # two heads' matmuls are emitted adjacently with disjoint PE row
        # groups (64-strips) so the array can overlap them.
        for r in range(JT):
            pss = {h2: ps_s.tile([128, 1024], F32, tag="ps_s",
                                 name=f"pss{t}_{r}_{h2}")
                   for h2 in range(2)}
            for i in range(2):
                m = 2 * r + i
                for h2 in range(2):
                    lo = h2 * 64
                    nc.tensor.matmul(
                        pss[h2][:, ds(i * 512, 512)],
                        K_sb[t][ds(lo, 64), ds(m * 128, 128)],
                        Q_sb[t][ds(lo, 64), :],
                        start=True, stop=True)
            for h2 in range(2):
                p = p_P.tile([128, 2, 512], FP8, tag="P")
                nc.scalar.activation(
                    p.rearrange("p i f -> p (i f)"), pss[h2], AF.Exp)
                P_sb[(t, h2, r)] = p

    def emit_pv(t):
        # PV with the ones-column denominator in psum row 64. ACT computes
        # 1/den as Exp(-Ln(den)) straight off the psum row; O is evacuated
        # to bf16 immediately (frees the bank) and normalized in place after
        # the reciprocal rows bounce back as a [128,512] broadcast.
        ot = p_O.tile([128, NQ], BF16, tag="O")
        drr = p_dram.tile([2, 512], F32, tag="dscratch", name=f"drr{t}")
        rep = p_rep.tile([128, 512], F32, tag="rep")
        for h2 in range(2):
            head = 2 * t + h2
            ps = ps_o.tile([65, 512], F32, tag="ps_o")
            for r in range(JT):
                nc.tensor.matmul(
                    ps, V2[r][:, :, head, :], P_sb[(t, h2, r)],
                    start=(r == 0), stop=(r == JT - 1),
                    perf_mode=DR)
            lnr = p_rq.tile([1, 512], F32, tag="lnr")
            nc.scalar.activation(lnr, ps[ds(64, 1), :], AF.Ln)
            nc.vector.tensor_copy(ot[ds(h2 * 64, 64), :], ps[ds(0, 64), :])
            rcp = p_rq.tile([1, 512], F32, tag="rcp")
            nc.scalar.activation(rcp, lnr, AF.Exp, scale=-1.0)
            nc.gpsimd.dma_start(drr[ds(h2, 1), :], rcp)
            nc.gpsimd.dma_start(rep[ds(h2 * 64, 64), :],
                                drr[ds(h2, 1), :].to_broadcast((64, 512)))
        nc.vector.tensor_mul(ot, ot, rep)
        O_sb.append(ot)

    w1_groups, w2_tiles = {}, {}
    while kq_early:
        kq_early.pop(0)()
    for t in range(KT):
        if t >= 2:
            emit_kq(t)
        emit_st(t)
        if t >= 1:
            emit_pv(t - 1)
        if 3 <= t <= 7:
            # fc1/fc2 leading weights: 1MB per head pair, anchored on the
            # previous O tile so each burst lands inside attention matmuls
            anchor = O_sb[t - 3][ds(0, 1), ds(0, 1)]
            i = t - 3
            if i < 3:
                w1_groups[i] = p_w1.tile([128, 4 * C], BF16, tag="w1",
                                         name=f"w1g{i}")
                nc.vector.tensor_copy(
                    w1_groups[i][ds(0, 1), ds(0, 1)], anchor)
                nc.sync.dma_start(w1_groups[i], w1g[i, :, :])
            else:
                w2_tiles[i - 3] = p_w2.tile([128, HID], BF16, tag="w2",
                                            name=f"w2p{i-3}")
                nc.vector.tensor_copy(
                    w2_tiles[i - 3][ds(0, 1), ds(0, 1)], anchor)
                nc.sync.dma_start(w2_tiles[i - 3], w2_m[i - 3, :, :])
    ps_s.release()   # frees 4 banks before the last PV/den tail
    emit_pv(KT - 1)
    for p in (p_rep, p_rq, p_P, p_Q, p_K, p_wkq):
        p.release()
    for p in (ps_o, ps_kq):
        p.release()

    # ---------------- proj + residual + LN2 stats ----------------
    p_sq2 = tc.alloc_tile_pool(name="sq2", bufs=3)
    ps_st2 = tc.alloc_tile_pool(name="ps_st2", bufs=1, space="PSUM")
    # pad pool pushes ps_p onto the banks the early ps_s release freed, so
    # proj matmuls for k<7 can run during the final PV/denominator chain
    ps_pad = tc.alloc_tile_pool(name="ps_pad", bufs=1, space="PSUM")
    ps_pad.tile([128, 512], F32, tag="pad0", name="pad0")
    ps_pad.tile([128, 512], F32, tag="pad1", name="pad1")
    ps_p = tc.alloc_tile_pool(name="ps_p", bufs=3, space="PSUM")

    # prefetch the first fc1 weight group + fc2 slice during proj


    ms2 = ps_st2.tile([1, 512], F32, tag="ms2")
    ss2 = ps_st2.tile([1, 512], F32, tag="ss2")
    x2 = []
    for m in range(KT):
        ps = ps_p.tile([128, 512], F32, tag="ps_p")
        for k in range(KT):
            nc.tensor.matmul(
                ps, wp_all[:, ds(m * C + k * 128, 128)], O_sb[k],
                start=(k == 0), stop=(k == KT - 1 and not has_bias["proj"]))
        if has_bias["proj"]:
            nc.tensor.matmul(ps, bias_sb[:, ds(bp_of + m * 128, 128)],
                             ones_r16, start=False, stop=True)
        xm = p_x2.tile([128, NQ], F32, tag="x2")
        nc.vector.tensor_add(xm, ps, xt[:, ds(m * NQ, NQ)])
        x2.append(xm)
        xb2 = p_sq2.tile([128, NQ], BF16, tag="xb2")
        nc.vector.tensor_copy(xb2, xm)
        sq = p_sq2.tile([128, NQ], BF16, tag="sq2")
        if m % 3 == 0:
            nc.gpsimd.tensor_mul(sq, xb2, xb2)
        elif m % 3 == 1:
            nc.scalar.activation(sq, xm, AF.Square)
        else:
            nc.vector.tensor_mul(sq, xb2, xb2)
        nc.tensor.matmul(ms2, ones_c, xb2,
                         start=(m == 0), stop=(m == KT - 1))
        nc.tensor.matmul(ss2, ones_c, sq,
                         start=(m == 0), stop=(m == KT - 1))

    for p in (p_sq2, p_O, p_h1, p_wv, p_V, p_wp, p_xT):
        p.release()
    ps_p.release()
    ps_pad.release()

    # ---------------- LN2 ----------------
    p_ln2 = tc.alloc_tile_pool(name="ln2", bufs=1)
    p_h2 = tc.alloc_tile_pool(name="h2", bufs=KT)
    rstd2_rep, musc2_rep = ln_chain([ms2], [ss2], NQ, p_ln2, "ln2",
                                    ps_warm=ps_st2, wtag="w2t")
    ps_st2.release()

    p_tmp2 = tc.alloc_tile_pool(name="tmp2", bufs=6)
    h2t = []
    for k in range(KT):
        eng = nc.gpsimd if k % 3 == 2 else nc.vector
        tmp = p_tmp2.tile([128, NQ], F32, tag="tmp2")
        eng.tensor_mul(tmp, x2[k], rstd2_rep)
        h = p_h2.tile([128, NQ], BF16, tag="h2")
        eng.tensor_sub(h, tmp, musc2_rep)
        h2t.append(h)
    p_tmp2.release()

    # ---------------- MLP (bf16) ----------------
    p_g = tc.alloc_tile_pool(name="g", bufs=MT1)
    p_y = tc.alloc_tile_pool(name="y", bufs=3)
    ps_m = tc.alloc_tile_pool(name="ps_m", bufs=8, space="PSUM")



    g_sb = []
    # first 8 fc1 output tiles k-outer: overlaps the h2 normalize
    ps8 = [ps_m.tile([128, 512], F32, tag="ps_m", name=f"ps8_{m}")
           for m in range(8)]
    for k in range(KT):
        for m in range(8):
            nc.tensor.matmul(
                ps8[m], w1_groups[m // 4][:, ds((m % 4) * C + k * 128, 128)],
                h2t[k],
                start=(k == 0), stop=(k == KT - 1 and not has_bias["fc1"]))
    for m in range(8):
        if has_bias["fc1"]:
            nc.tensor.matmul(ps8[m], bias_sb[:, ds(b1_of + m * 128, 128)],
                             ones_r16, start=False, stop=True)
        g = p_g.tile([128, NQ], BF16, tag="g")
        nc.scalar.activation(g, ps8[m], gelu_func)
        g_sb.append(g)
    for m in range(8, MT1):
        gi = m // 4
        la = gi + 1  # one-group lookahead
        if la < MT1 // 4 and la not in w1_groups:
            w1_groups[la] = p_w1.tile([128, 4 * C], BF16, tag="w1",
                                      name=f"w1g{la}")
            nc.sync.dma_start(w1_groups[la], w1g[la, :, :])
        ps = ps_m.tile([128, 512], F32, tag="ps_m")
        for k in range(KT):
            nc.tensor.matmul(
                ps, w1_groups[gi][:, ds((m % 4) * C + k * 128, 128)], h2t[k],
                start=(k == 0), stop=(k == KT - 1 and not has_bias["fc1"]))
        if has_bias["fc1"]:
            nc.tensor.matmul(ps, bias_sb[:, ds(b1_of + m * 128, 128)],
                             ones_r16, start=False, stop=True)
        g = p_g.tile([128, NQ], BF16, tag="g")
        nc.scalar.activation(g, ps, gelu_func)
        g_sb.append(g)

    for m in range(KT):
        la = m + 2
        if la < KT and la not in w2_tiles:
            w2_tiles[la] = p_w2.tile([128, HID], BF16, tag="w2",
                                     name=f"w2p{la}")
            nc.sync.dma_start(w2_tiles[la], w2_m[la, :, :])
        w2t = w2_tiles[m]
        ps = ps_m.tile([128, 512], F32, tag="ps_m")
        for k in range(MT1):
            nc.tensor.matmul(
                ps, w2t[:, ds(k * 128, 128)], g_sb[k],
                start=(k == 0), stop=(k == MT1 - 1 and not has_bias["fc2"]))
        if has_bias["fc2"]:
            nc.tensor.matmul(ps, bias_sb[:, ds(b2_of + m * 128, 128)],
                             ones_r16, start=False, stop=True)
        y = p_y.tile([128, NQ], F32, tag="y")
        nc.vector.tensor_add(y, ps, x2[m])
        nc.sync.dma_start(yT[ds(m * 128, 128), :], y)

    for p in (p_y, p_g, p_h2, p_ln2, p_w1, p_w2, p_x2, pers):
        p.release()
    ps_m.release()
    p_dram.release()


# --------------------------------------------------------------------------
# Host side
# --------------------------------------------------------------------------
def _m_slice(w, mtiles):
    """[K_in, M_out] -> [mtiles, 128, K_in] with free dim k-major."""
    kin = w.shape[0]
    kt = kin // 128
    a = w.reshape(kt, 128, mtiles, 128)        # [k, i, m, j]
    return np.ascontiguousarray(a.transpose(2, 1, 0, 3).reshape(mtiles, 128, kin))


def _pair_m(w, mtiles):
    """fp8 DoubleRow stationary layout for out^T = w^T @ act:
    out[t, p, j*256 + i*128 + f] = w[(2j+i)*128 + p, t*128 + f]."""
    kin = w.shape[0]
    jt = kin // 256
    a = w.reshape(jt, 2, 128, mtiles, 128)     # [j, i, p, t, f]
    return np.ascontiguousarray(
        a.transpose(3, 2, 0, 1, 4).reshape(mtiles, 128, jt * 256))


def _pair_r(w):
    """fp8 DoubleRow moving layout, packed: [128, JT*2*F]:
    out[p, j*2F + i*F + f] = w[(2j+i)*128+p, f]."""
    kin, f = w.shape
    jt = kin // 256
    a = w.reshape(jt, 2, 128, f)               # [j, i, p, f]
    return np.ascontiguousarray(a.transpose(2, 0, 1, 3).reshape(128, jt * 2 * f))


def _feat_pack(xt, n):
    """[C, n] -> [128, KT*n]: out[p, k*n + c] = xt[k*128 + p, c]."""
    return np.ascontiguousarray(
        xt.reshape(KT, 128, n).transpose(1, 0, 2).reshape(128, KT * n))


def _prep(inputs):
    f32 = np.float32
    x = np.asarray(inputs["x"], f32)
    ln1_g = np.asarray(inputs["ln1_g"], f32)
    ln1_b = np.asarray(inputs["ln1_b"], f32)
    ln2_g = np.asarray(inputs["ln2_g"], f32)
    ln2_b = np.asarray(inputs["ln2_b"], f32)
    w_qkv = np.asarray(inputs["w_qkv"], f32)
    w_proj = np.asarray(inputs["w_proj"], f32)
    w_fc1 = np.asarray(inputs["w_fc1"], f32)
    w_fc2 = np.asarray(inputs["w_fc2"], f32)

    wqkv_e = ln1_g[:, None] * w_qkv
    bqkv_e = ln1_b @ w_qkv + np.asarray(inputs["b_qkv"], f32)
    wfc1_e = ln2_g[:, None] * w_fc1
    bfc1_e = ln2_b @ w_fc1 + np.asarray(inputs["b_fc1"], f32)
    b_proj = np.asarray(inputs["b_proj"], f32)
    b_fc2 = np.asarray(inputs["b_fc2"], f32)

    bf = ml_dtypes.bfloat16
    f8 = ml_dtypes.float8_e4m3
    wq, wk, wvv = wqkv_e[:, :C], wqkv_e[:, C:2 * C], wqkv_e[:, 2 * C:]

    def q8(a):
        return np.clip(a * WSCALE, -240, 240).astype(f8)

    w1s = _m_slice(wfc1_e, MT1)                 # [32, 128, C]
    shared = {
        "wkq8": np.concatenate(
            [q8(_pair_m(wk, KT)), q8(_pair_m(wq, KT))], axis=2),
        "wv8": q8(_pair_r(wvv)),
        "wpp": np.ascontiguousarray(
            _m_slice(w_proj, KT).transpose(1, 0, 2).reshape(128, KT * C)
        ).astype(bf),
        "w1g": np.ascontiguousarray(
            w1s.reshape(MT1 // 4, 4, 128, C).transpose(0, 2, 1, 3)
            .reshape(MT1 // 4, 128, 4 * C)).astype(bf),
        "w2_m": _m_slice(w_fc2, KT).astype(bf),
        # qkv biases ride the fp8-scaled psum, so pre-scale them by WSCALE
        "b_all": np.concatenate(
            [bqkv_e * WSCALE, b_proj, bfc1_e, b_fc2])[None, :].astype(bf),
    }
    has_bias = {
        "qk": bool(np.any(bqkv_e[:2 * C])),
        "v": bool(np.any(bqkv_e[2 * C:])),
        "proj": bool(np.any(b_proj)),
        "fc1": bool(np.any(bfc1_e)),
        "fc2": bool(np.any(b_fc2)),
    }

    in_maps = []
    for c in range(8):
        b, half = c // 2, c % 2
        xb = x[b]
        if half:
            xb = np.concatenate([xb[NQ:], xb[:NQ]], axis=0)
        xt = np.ascontiguousarray(xb.T)
        m = {"xTp": _feat_pack(xt[:, :NQ], NQ),
             "xbp": _feat_pack(xt, NTOK).astype(bf),
             **shared}
        in_maps.append(m)
    return in_maps, has_bias


def kernel(**inputs):
    in_maps, has_bias = _prep(inputs)
    key = tuple(sorted(has_bias.items()))
    if key not in _cache:
        nc = build_program(has_bias)
        _split_wide_waits(nc, 1)
        _cache[key] = nc
    nc = _cache[key]

    res = bass_utils.run_bass_kernel_spmd(
        nc, in_maps, core_ids=list(range(8)), trace=False)

    x = np.asarray(inputs["x"])
    out = np.empty((4, NTOK, C), dtype=np.float32)
    for c in range(8):
        b, half = c // 2, c % 2
        out[b, half * NQ:(half + 1) * NQ, :] = res.results[c]["yT"].T
    return out.astype(x.dtype, copy=False)



# revision 17
# speedup vs baseline: 1.1133x; 1.1133x over previous
"""Trainium2 Bass kernel for a pre-norm transformer block (nn_Block).

Math (per batch b of x [4, 1024, 1024]):
    h  = LN(x) ; qkv = h @ w_qkv + b_qkv ; attention (16 heads, dh=64)
    x  = x + (attn_out @ w_proj + b_proj)
    h  = LN(x) ; x = x + gelu(h @ w_fc1 + b_fc1) @ w_fc2 + b_fc2

Sharding: communication-free hybrid over 8 cores. Core c handles batch
b = c // 2 and query-token half c % 2. Each core computes K and V for its
batch's full 1024 tokens and everything else for its own 512 queries.

Precision: K/Q/V, S^T, exp(P), PV, proj and fc2 run in fp8-e4m3 with
DoubleRow (2 contraction blocks per matmul, ~2x); fc1 stays bf16 (fp8 on
both fc matmuls measured 2.4e-2 end-to-end, over the 2e-2 gate; fc2-only
measured ~1.7e-2).

vs the previous 344us version (trace-driven):
  - softmax exp was the serializer (64 ACT EXPs ~1.15us each gating the
    S^T psum ping-pong; PE starved in 1-1.7us gaps and HAM re-throttled
    it to 1.2GHz for ~60us). Exps now split: ACT keeps half, the vector
    engine computes the rest as Schraudolph exp (i32 = A*S + B via
    tensor_scalar convert, bitcast back to f32, copy to fp8; rms err
    ~1.8% vs e4m3's 3.6% quantization - end-to-end delta +1e-5).
  - LN resolve used a DRAM-bounce row broadcast (8.5us dead latency) and
    mixed DVE/GpSimd normalize (SBUF port collisions tripled op time).
    Rows now broadcast via a rank-1 PE matmul (ones[1,128] x row[1,512]
    -> psum) and normalize runs DVE-only on bf16.
  - PV softmax denominators ride a 1/16-column through the fp8 PV psum;
    ACT's Exp(-Ln(den/16)) = 16/den is exactly the fp8 O scale; the
    per-head-pair reciprocal rows broadcast through the same PE trick.
  - proj/fc2 weights host-packed into DoubleRow pair layout; O and gelu
    outputs written as fp8 pair tiles; psum scales folded into the
    residual-add (scalar_tensor_tensor) evicts.
  - K/Q production all happens inside the V phase (frees 2 psum banks ->
    S^T runs a 2x[128,1024] ping-pong + 3 PV banks + 1 broadcast bank).
  - dummy matmuls chained on the LN row chain keep the PE HAM warm
    across the two LN windows.
"""

import os
import sys

import numpy as np

try:
    import concourse.bass as bass
except ImportError:  # pragma: no cover
    for _p in ("/opt/trn_rl_repo", "/root/.axon_site/_ro/trn_rl_repo"):
        if os.path.isdir(_p) and _p not in sys.path:
            sys.path.insert(0, _p)
    import concourse.bass as bass

import ml_dtypes
import concourse.tile as tile
import concourse.mybir as mybir
from concourse import bass_utils
from concourse.bass import ds

F32 = mybir.dt.float32
BF16 = mybir.dt.bfloat16
FP8 = mybir.dt.float8e4
I32 = mybir.dt.int32
AF = mybir.ActivationFunctionType
ALU = mybir.AluOpType
DR = mybir.MatmulPerfMode.DoubleRow

C = 1024          # model dim
H = 16            # heads
DH = 64           # head dim
NTOK = 1024       # tokens per batch (keys/values)
NQ = 512          # query tokens per core
KT = C // 128     # 8 feature tiles
JT = KT // 2      # 4 feature-pair tiles (DoubleRow)
HID = 4096
MT1 = HID // 128  # 32 fc1 output tiles
JT2 = HID // 256  # 16 fc2 contraction pairs
EPS = 1e-5
WSCALE = 2048.0   # pow2 scale folded into fp8 weights
QSM = 1.0 / (WSCALE * float(DH) ** 0.5)   # Q psum -> fp8 cast scale
KSM = 1.0 / WSCALE                        # K/V psum -> fp8 cast scale
OSC = 16.0                                # fp8 O scale (from 1/16 ones col)
PSM = 1.0 / (OSC * WSCALE)                # proj psum -> f32 scale
Y2M = 1.0 / WSCALE                        # fc2 psum -> f32 scale

EXP_A = float(2 ** 23 / np.log(2.0))
EXP_B = float(127 * 2 ** 23 - 486411)     # RMS-optimal Schraudolph offset

# exp engine per (r, h2): 'A' ACT table exp; 'D' DVE schraudolph;
# 'G' DVE op1 + GpSimd op2
EXP_ENG = {(r, h2): ('A' if h2 == 0 else ('G' if r == 3 else 'D'))
           for r in range(JT) for h2 in range(2)}

_cache = {}


def _split_wide_waits(nc, max_waits=1):
    """Walrus on this image rejects instructions carrying more than one
    semaphore wait; split the excess onto same-engine NOPs."""
    ctr = 0
    for f in nc.m.functions:
        for b in f.blocks:
            out, changed = [], False
            for inst in b.instructions:
                si = getattr(inst, "sync_info", None)
                if si is not None and si.on_wait and len(si.on_wait) > max_waits:
                    waits = list(si.on_wait)
                    extra, keep = waits[:-max_waits], waits[-max_waits:]
                    for gs in range(0, len(extra), max_waits):
                        ctr += 1
                        nop = mybir.InstNoOp(
                            name=f"waitsplit-{ctr}", ins=[], outs=[])
                        nop.engine = inst.engine
                        nop.sync_info = mybir.SyncInfo(
                            on_wait=extra[gs:gs + max_waits], on_update=[])
                        out.append(nop)
                    inst.sync_info = mybir.SyncInfo(
                        on_wait=keep, on_update=list(si.on_update))
                    changed = True
                out.append(inst)
            if changed:
                b.instructions = out


def build_program(has_bias, gelu_func=None):
    nc = bass.Bass()

    xTp = nc.dram_tensor("xTp", [128, KT * NQ], F32, kind="ExternalInput")
    xbp = nc.dram_tensor("xbp", [128, KT * NTOK], BF16, kind="ExternalInput")
    wkq8 = nc.dram_tensor("wkq8", [KT, 128, 2 * C], FP8, kind="ExternalInput")
    wv8 = nc.dram_tensor("wv8", [128, JT * 2 * C], FP8, kind="ExternalInput")
    wpDR = nc.dram_tensor("wpDR", [KT, 128, C], FP8, kind="ExternalInput")
    sel2in = nc.dram_tensor("sel2in", [2, 128], BF16, kind="ExternalInput")
    w1g = nc.dram_tensor("w1g", [MT1 // 4, 128, 4 * C], BF16,
                         kind="ExternalInput")
    w2DR = nc.dram_tensor("w2DR", [KT, 128, HID], FP8, kind="ExternalInput")
    b_all = nc.dram_tensor("b_all", [1, 3 * C + C + HID + C], BF16,
                           kind="ExternalInput")
    yT = nc.dram_tensor("yT", [C, NQ], F32, kind="ExternalOutput")

    with tile.TileContext(nc) as tc:
        _emit(nc, tc, xTp, xbp, wkq8, wv8, wpDR, w1g, w2DR, b_all,
              sel2in, yT, has_bias, gelu_func or AF.Gelu)
    return nc


def _emit(nc, tc, xTp, xbp, wkq8, wv8, wpDR, w1g, w2DR, b_all,
          sel2in, yT, has_bias, gelu_func):
    pers = tc.alloc_tile_pool(name="pers", bufs=1)
    ones_c = pers.tile([128, 1], BF16, tag="ones_c")      # stats lhsT
    nc.vector.memset(ones_c, 1.0)
    ones_r16 = pers.tile([1, NQ], BF16, tag="ones_r16")   # bias rank-1 rhs
    nc.vector.memset(ones_r16, 1.0)
    ones_tok16 = pers.tile([1, 128], BF16, tag="ones_tok16")  # v-bias lhsT
    nc.vector.memset(ones_tok16, 1.0)
    ones_b = pers.tile([1, 128], BF16, tag="ones_b")      # broadcast lhsT
    nc.vector.memset(ones_b, 1.0)
    sel2 = pers.tile([2, 128], BF16, tag="sel2")          # 2-head bcast lhsT
    nc.sync.dma_start(sel2, sel2in[:, :])
    eps_t = pers.tile([128, 1], F32, tag="eps_t")
    nc.vector.memset(eps_t, EPS)
    junk_l = pers.tile([1, 128], BF16, tag="junk_l")      # keep-warm lhsT
    nc.vector.memset(junk_l, 0.0)

    any_bias = any(has_bias.values())
    if any_bias:
        bias_sb = pers.tile([1, 3 * C + C + HID + C], BF16, tag="bias_sb")
        nc.sync.dma_start(bias_sb, b_all[:])
        bq_of, bk_of, bv_of = 0, C, 2 * C
        bp_of, b1_of, b2_of = 3 * C, 4 * C, 4 * C + HID

    # pools ordered by lifetime (latest-dying first): releases are a
    # strict LIFO stack per memory space
    p_x2 = tc.alloc_tile_pool(name="x2", bufs=KT)
    p_w2 = tc.alloc_tile_pool(name="w2", bufs=3)
    p_w1 = tc.alloc_tile_pool(name="w1", bufs=3)
    p_xb2 = tc.alloc_tile_pool(name="xb2", bufs=KT)
    p_xT = tc.alloc_tile_pool(name="xT", bufs=1)
    p_wp = tc.alloc_tile_pool(name="wp", bufs=1)
    p_O = tc.alloc_tile_pool(name="O", bufs=JT)
    p_sq2 = tc.alloc_tile_pool(name="sq2", bufs=3)
    p_V = tc.alloc_tile_pool(name="V", bufs=JT)
    p_K = tc.alloc_tile_pool(name="K", bufs=KT)
    p_Q = tc.alloc_tile_pool(name="Q", bufs=KT)
    p_h1 = tc.alloc_tile_pool(name="h1", bufs=JT)
    p_wv = tc.alloc_tile_pool(name="wv", bufs=1)
    p_wkq = tc.alloc_tile_pool(name="wkq", bufs=4)
    p_xb = tc.alloc_tile_pool(name="xb", bufs=1)
    p_ln1 = tc.alloc_tile_pool(name="ln1", bufs=1)
    p_sq = tc.alloc_tile_pool(name="sq", bufs=3)
    ps_stat = tc.alloc_tile_pool(name="ps_stat", bufs=1, space="PSUM")
    ps_lnb = tc.alloc_tile_pool(name="ps_lnb", bufs=2, space="PSUM")

    warm_n = [0]

    def keep_warm(warm_ps, dep_row, n=3):
        """Chain n dummy matmuls on dep_row (a [1,>=1] tile AP) so the PE
        HAM sees activity while the LN row chain resolves."""
        s = warm_n[0] * 3 % 96
        warm_n[0] += 1
        nc.vector.tensor_copy(junk_l[:, ds(s, 1)], dep_row[:, ds(0, 1)])
        for i in range(n):
            nc.tensor.matmul(warm_ps, junk_l, ones_r16, start=True, stop=True)

    def ln_chain(ms, ss, N, pool, pspool, warmpool, nm):
        """From per-chunk sum/sumsq psum rows produce [128, N] bf16
        rstd_rep and (mu*rstd)_rep via rank-1 PE broadcasts."""
        nch = N // 512
        warm_ps = warmpool.tile([128, NQ], F32, tag="warm", name=f"warm{nm}")
        rrep = pool.tile([128, N], BF16, tag=f"rrep_{nm}", name=f"rrep_{nm}")
        mrep = pool.tile([128, N], BF16, tag=f"mrep_{nm}", name=f"mrep_{nm}")
        for n in range(nch):
            tn = pool.tile([1, 512], F32, tag=f"tn_{nm}", name=f"tn_{nm}{n}")
            nc.scalar.activation(tn, ms[n], AF.Square)
            keep_warm(warm_ps, tn)
            vn = pool.tile([1, 512], F32, tag=f"vn_{nm}", name=f"vn_{nm}{n}")
            nc.vector.scalar_tensor_tensor(
                vn, tn, 1.0 / C, ss[n], ALU.mult, ALU.subtract)
            lnv = pool.tile([1, 512], F32, tag=f"lnv_{nm}",
                            name=f"lnv_{nm}{n}")
            nc.scalar.activation(lnv, vn, AF.Ln, bias=eps_t[ds(0, 1), :],
                                 scale=-1.0 / C)
            keep_warm(warm_ps, lnv)
            rsr = pool.tile([1, 512], BF16, tag=f"rsr_{nm}",
                            name=f"rsr_{nm}{n}")
            nc.scalar.activation(rsr, lnv, AF.Exp, scale=-0.5)
            msr = pool.tile([1, 512], BF16, tag=f"msr_{nm}",
                            name=f"msr_{nm}{n}")
            nc.vector.scalar_tensor_tensor(
                msr, ms[n], 1.0 / C, rsr, ALU.mult, ALU.mult)
            keep_warm(warm_ps, msr)
            rst_ps = pspool.tile([128, 512], F32, tag="lnb",
                                 name=f"rst_ps{nm}{n}")
            nc.tensor.matmul(rst_ps, ones_b, rsr, start=True, stop=True)
            nc.vector.tensor_copy(rrep[:, ds(n * 512, 512)], rst_ps)
            mus_ps = pspool.tile([128, 512], F32, tag="lnb",
                                 name=f"mus_ps{nm}{n}")
            nc.tensor.matmul(mus_ps, ones_b, msr, start=True, stop=True)
            nc.vector.tensor_copy(mrep[:, ds(n * 512, 512)], mus_ps)
        return rrep, mrep

    # ---- bulk loads: one big DMA each, SBUF views per tile ----
    xb_all = p_xb.tile([128, KT * NTOK], BF16, tag="xb")
    for q in range(4):
        nc.sync.dma_start(xb_all[:, ds(q * 2 * NTOK, 2 * NTOK)],
                          xbp[:, ds(q * 2 * NTOK, 2 * NTOK)])
    xbt = [xb_all[:, ds(k * NTOK, NTOK)] for k in range(KT)]

    wv_all = p_wv.tile([128, JT * 2 * C], FP8, tag="wv")
    nc.sync.dma_start(wv_all, wv8[:, :])
    wv = [wv_all[:, ds(j * 2 * C, 2 * C)] for j in range(JT)]

    # prefetch K/Q weights for the first head pair only
    wkq_tiles = {}
    for t0 in range(2):
        w = p_wkq.tile([128, 2 * C], FP8, tag="wkq", name=f"wkq{t0}")
        nc.sync.dma_start(w, wkq8[t0, :, :])
        wkq_tiles[t0] = w

    # V2[r]: pair layout [128 tok, i(2), H, 65] fp8; [.., 64] is the 1/16 col
    V2 = []
    for r in range(JT):
        vt = p_V.tile([128, 2, H, 65], FP8, tag="V", name=f"V{r}")
        nc.vector.memset(vt[:, :, :, ds(64, 1)], 1.0 / OSC)
        V2.append(vt)

    # ---- LN1 stats ----
    ms = [ps_stat.tile([1, 512], F32, tag=f"ms{n}", name=f"ms{n}")
          for n in range(2)]
    ss = [ps_stat.tile([1, 512], F32, tag=f"ss{n}", name=f"ss{n}")
          for n in range(2)]
    for k in range(KT):
        sq = p_sq.tile([128, NTOK], BF16, tag="sq")
        if k % 4 == 2:
            nc.scalar.activation(sq, xbt[k], AF.Square)
        else:
            nc.vector.tensor_mul(sq, xbt[k], xbt[k])
        for n in range(2):
            nc.tensor.matmul(ms[n], ones_c, xbt[k][:, ds(n * 512, 512)],
                             start=(k == 0), stop=(k == KT - 1))
            nc.tensor.matmul(ss[n], ones_c, sq[:, ds(n * 512, 512)],
                             start=(k == 0), stop=(k == KT - 1))
    p_sq.release()

    rstd_rep, musc_rep = ln_chain(ms, ss, NTOK, p_ln1, ps_lnb, ps_stat, "ln1")

    # h1p[j]: fp8 pair tile [128, 2, NTOK]; halves are feature blocks 2j,2j+1
    p_tmp = tc.alloc_tile_pool(name="tmp", bufs=3)
    h1 = [p_h1.tile([128, 2, NTOK], FP8, tag="h1", name=f"h1p{j}")
          for j in range(JT)]
    for k in range(KT):
        tmp = p_tmp.tile([128, NTOK], F32, tag="tmp")
        nc.vector.tensor_mul(tmp, xbt[k], rstd_rep)
        nc.vector.tensor_sub(h1[k // 2][:, k % 2, :], tmp, musc_rep)
    p_tmp.release()

    # bulk proj-phase loads anchored on the last h1 tile so the bursts land
    # during the matmul-heavy V/attention phases
    xt = p_xT.tile([128, KT * NQ], F32, tag="xT")
    wp_all = p_wp.tile([128, KT * C], FP8, tag="wp")
    nc.vector.tensor_copy(xt[ds(0, 1), ds(0, 1)], h1[3][ds(0, 1), 1, ds(0, 1)])
    nc.sync.dma_start(xt, xTp[:, :])
    nc.vector.tensor_copy(wp_all[ds(0, 1), ds(0, 1)],
                          h1[3][ds(0, 1), 1, ds(0, 1)])
    for m in range(KT):
        nc.sync.dma_start(wp_all[:, ds(m * C, C)], wpDR[m, :, :])
    p_ln1.release()
    p_xb.release()
    ps_lnb.release()
    ps_stat.release()

    # ---------------- V + K + Q (token-major, DoubleRow) ----------------
    ps_kq = tc.alloc_tile_pool(name="ps_kq", bufs=2, space="PSUM")
    ps_v = tc.alloc_tile_pool(name="ps_v", bufs=6, space="PSUM")

    K_sb, Q_sb, P_sb, O_pair = [], [], {}, []

    def emit_kq(t):
        wt = wkq_tiles.pop(t)
        wkt, wqt = wt[:, ds(0, C)], wt[:, ds(C, C)]
        if t + 2 < KT:  # keep two pairs in flight
            nw = p_wkq.tile([128, 2 * C], FP8, tag="wkq", name=f"wkq{t+2}")
            nc.sync.dma_start(nw, wkq8[t + 2, :, :])
            wkq_tiles[t + 2] = nw
        kt_sb = p_K.tile([128, NTOK], FP8, tag="K")
        wkp = wkt.rearrange("p (j i f) -> p j i f", j=JT, i=2)
        for n in range(2):
            ps = ps_kq.tile([128, 512], F32, tag="ps_kq")
            for j in range(JT):
                nc.tensor.matmul(
                    ps, wkp[:, j],
                    h1[j][:, :, ds(n * 512, 512)],
                    start=(j == 0), stop=(j == JT - 1 and not has_bias["qk"]),
                    perf_mode=DR)
            if has_bias["qk"]:
                nc.tensor.matmul(
                    ps, bias_sb[:, ds(bk_of + t * 128, 128)], ones_r16,
                    start=False, stop=True)
            nc.vector.tensor_scalar_mul(kt_sb[:, ds(n * 512, 512)], ps, KSM)
        K_sb.append(kt_sb)

        qt_sb = p_Q.tile([128, NQ], FP8, tag="Q")
        wqp = wqt.rearrange("p (j i f) -> p j i f", j=JT, i=2)
        ps = ps_kq.tile([128, 512], F32, tag="ps_kq")
        for j in range(JT):
            nc.tensor.matmul(
                ps, wqp[:, j], h1[j][:, :, ds(0, 512)],
                start=(j == 0), stop=(j == JT - 1 and not has_bias["qk"]),
                perf_mode=DR)
        if has_bias["qk"]:
            nc.tensor.matmul(
                ps, bias_sb[:, ds(bq_of + t * 128, 128)], ones_r16,
                start=False, stop=True)
        nc.vector.tensor_scalar_mul(qt_sb, ps, QSM)
        Q_sb.append(qt_sb)

    # V in groups of 2 feature-tiles with all 8 K/Q pairs interleaved
    kq_queue = list(range(KT))
    for g0 in range(0, KT, 2):
        ts_ = range(g0, min(g0 + 2, KT))
        psv = {(t, n): ps_v.tile([128, 512], F32, tag="ps_v",
                                 name=f"psv{t}_{n}")
               for t in ts_ for n in range(2)}
        for j in range(JT):
            for t in ts_:
                for n in range(2):
                    nc.tensor.matmul(
                        psv[(t, n)], h1[j][:, :, ds(t * 128, 128)],
                        wv[j].rearrange("p (i f) -> p i f", i=2)[
                            :, :, ds(n * 512, 512)],
                        start=(j == 0),
                        stop=(j == JT - 1 and not has_bias["v"]),
                        perf_mode=DR)
        for t in ts_:
            for n in range(2):
                if has_bias["v"]:
                    nc.tensor.matmul(
                        psv[(t, n)], ones_tok16,
                        bias_sb[:, ds(bv_of + n * 512, 512)],
                        start=False, stop=True)
                nc.vector.tensor_scalar_mul(
                    V2[t // 2][:, t % 2, ds(n * 8, 8), ds(0, 64)],
                    psv[(t, n)].rearrange("p (h d) -> p h d", d=64), KSM)
        for _ in range(2):
            if kq_queue:
                emit_kq(kq_queue.pop(0))

    ps_v.release()
    ps_kq.release()
    p_wkq.release()
    p_wv.release()
    p_h1.release()

    # ---------------- attention: S^T + softmax + PV ---------------------
    p_P = tc.alloc_tile_pool(name="P", bufs=14)
    p_i32 = tc.alloc_tile_pool(name="i32", bufs=3)
    p_den = tc.alloc_tile_pool(name="den", bufs=3)
    p_rep = tc.alloc_tile_pool(name="rep", bufs=2)
    ps_s = tc.alloc_tile_pool(name="ps_s", bufs=2, space="PSUM")
    ps_pv = tc.alloc_tile_pool(name="ps_pv", bufs=3, space="PSUM")
    ps_rep = tc.alloc_tile_pool(name="ps_rep", bufs=1, space="PSUM")

    for j in range(JT):
        O_pair.append(p_O.tile([128, 2, NQ], FP8, tag="O", name=f"Op{j}"))

    def emit_st(t):
        for r in range(JT):
            pss = {h2: ps_s.tile([128, 2, 512], F32, tag="ps_s",
                                 name=f"pss{t}_{r}_{h2}")
                   for h2 in range(2)}
            for i in range(2):
                m = 2 * r + i
                for h2 in range(2):
                    lo = h2 * 64
                    nc.tensor.matmul(
                        pss[h2][:, i, :],
                        K_sb[t][ds(lo, 64), ds(m * 128, 128)],
                        Q_sb[t][ds(lo, 64), :],
                        start=True, stop=True)
            for h2 in range(2):
                p = p_P.tile([128, 2, 512], FP8, tag="P")
                flat_in = pss[h2].rearrange("p i f -> p (i f)")
                flat_out = p.rearrange("p i f -> p (i f)")
                eng = EXP_ENG[(r, h2)]
                if eng == 'A':
                    nc.scalar.activation(flat_out, flat_in, AF.Exp)
                else:
                    it = p_i32.tile([128, 1024], I32, tag="i32")
                    nc.vector.tensor_scalar(
                        it, flat_in, EXP_A, EXP_B, ALU.mult, ALU.add)
                    fview = it[:, :].bitcast(F32)
                    if eng == 'D':
                        nc.vector.tensor_copy(flat_out, fview)
                    else:
                        nc.gpsimd.tensor_copy(flat_out, fview)
                P_sb[(t, h2, r)] = p

    def emit_pv(t):
        # PV with the 1/16-column denominator in psum row 64. den rows ->
        # ACT Exp(-Ln) = 16/den -> rank-1 PE broadcast -> GpSimd evicts
        # O pair halves as (psum * rcp_rep) in fp8.
        den2 = p_den.tile([2, 512], BF16, tag="den", name=f"den{t}")
        pvs = {}
        for h2 in range(2):
            head = 2 * t + h2
            ps = ps_pv.tile([65, 512], F32, tag="ps_pv", name=f"pspv{t}_{h2}")
            for r in range(JT):
                nc.tensor.matmul(
                    ps, V2[r][:, :, head, :], P_sb[(t, h2, r)],
                    start=(r == 0), stop=(r == JT - 1),
                    perf_mode=DR)
            if h2 == 0:
                nc.vector.tensor_copy(den2[ds(0, 1), :], ps[ds(64, 1), :])
            else:
                dtmp = p_den.tile([1, 512], BF16, tag="dtmp",
                                  name=f"dtmp{t}")
                nc.vector.tensor_copy(dtmp, ps[ds(64, 1), :])
                nc.sync.dma_start(den2[ds(1, 1), :], dtmp)
            pvs[h2] = ps
        lnr = p_den.tile([2, 512], F32, tag="lnr", name=f"lnr{t}")
        nc.scalar.activation(lnr, den2, AF.Ln)
        rcp2 = p_den.tile([2, 512], BF16, tag="rcp", name=f"rcp{t}")
        nc.scalar.activation(rcp2, lnr, AF.Exp, scale=-1.0)
        rps = ps_rep.tile([128, 512], F32, tag="ps_rep", name=f"rps{t}")
        nc.tensor.matmul(rps, sel2, rcp2, start=True, stop=True)
        rep = p_rep.tile([128, 512], BF16, tag="rep")
        nc.vector.tensor_copy(rep, rps)
        # h2=0: DVE multiplies straight from psum; h2=1: DVE evicts to
        # bf16 and GpSimd (no PSUM access) does the multiply in SBUF
        nc.vector.tensor_mul(
            O_pair[t // 2][ds(0, 64), t % 2, :],
            pvs[0][ds(0, 64), :], rep[ds(0, 64), :])
        otb = p_rep.tile([128, 512], BF16, tag="otb", name=f"otb{t}")
        nc.vector.tensor_copy(otb[ds(64, 64), :], pvs[1][ds(0, 64), :])
        nc.gpsimd.tensor_mul(
            O_pair[t // 2][ds(64, 64), t % 2, :],
            otb[ds(64, 64), :], rep[ds(64, 64), :])

    w1_groups, w2_tiles = {}, {}
    for t in range(KT):
        emit_st(t)
        if t >= 1:
            emit_pv(t - 1)
        if 3 <= t <= 7:
            # fc1/fc2 weights: anchored so each burst lands inside the
            # attention matmul phase
            anchor = O_pair[(t - 3) // 2][ds(0, 1), (t - 3) % 2, ds(0, 1)]
            i = t - 3
            if i < 3:
                w1_groups[i] = p_w1.tile([128, 4 * C], BF16, tag="w1",
                                         name=f"w1g{i}")
                nc.vector.tensor_copy(
                    w1_groups[i][ds(0, 1), ds(0, 1)], anchor)
                nc.sync.dma_start(w1_groups[i], w1g[i, :, :])
            else:
                w2_tiles[i - 3] = p_w2.tile([128, HID], FP8, tag="w2",
                                            name=f"w2p{i-3}")
                nc.vector.tensor_copy(
                    w2_tiles[i - 3][ds(0, 1), ds(0, 1)], anchor)
                nc.sync.dma_start(w2_tiles[i - 3], w2DR[i - 3, :, :])
    emit_pv(KT - 1)
    for p in (p_rep, p_den, p_i32, p_P, p_Q, p_K, p_V):
        p.release()
    for p in (ps_rep, ps_pv, ps_s):
        p.release()

    # ---------------- proj (fp8 DR) + residual + LN2 stats ----------------
    ps_st2 = tc.alloc_tile_pool(name="ps_st2", bufs=1, space="PSUM")
    ps_ln2b = tc.alloc_tile_pool(name="ps_ln2b", bufs=2, space="PSUM")
    ps_p = tc.alloc_tile_pool(name="ps_p", bufs=3, space="PSUM")

    ms2 = ps_st2.tile([1, 512], F32, tag="ms2")
    ss2 = ps_st2.tile([1, 512], F32, tag="ss2")
    x2, xb2t = [], []
    for m in range(KT):
        ps = ps_p.tile([128, 512], F32, tag="ps_p")
        wpp_m = wp_all[:, ds(m * C, C)]
        for j in range(JT):
            nc.tensor.matmul(
                ps, wpp_m[:, ds(j * 256, 256)].rearrange(
                    "p (i f) -> p i f", i=2),
                O_pair[j],
                start=(j == 0), stop=(j == JT - 1 and not has_bias["proj"]),
                perf_mode=DR)
        if has_bias["proj"]:
            nc.tensor.matmul(ps, bias_sb[:, ds(bp_of + m * 128, 128)],
                             ones_r16, start=False, stop=True)
        xm = p_x2.tile([128, NQ], F32, tag="x2")
        nc.vector.scalar_tensor_tensor(
            xm, ps, PSM, xt[:, ds(m * NQ, NQ)], ALU.mult, ALU.add)
        x2.append(xm)
        xb2 = p_xb2.tile([128, NQ], BF16, tag="xb2", name=f"xb2_{m}")
        nc.vector.tensor_copy(xb2, xm)
        xb2t.append(xb2)
        sq = p_sq2.tile([128, NQ], BF16, tag="sq2")
        if m % 2 == 1:
            nc.scalar.activation(sq, xm, AF.Square)
        else:
            nc.vector.tensor_mul(sq, xb2, xb2)
        nc.tensor.matmul(ms2, ones_c, xb2,
                         start=(m == 0), stop=(m == KT - 1))
        nc.tensor.matmul(ss2, ones_c, sq,
                         start=(m == 0), stop=(m == KT - 1))

    p_sq2.release()
    for p in (p_O, p_wp, p_xT):
        p.release()
    ps_p.release()

    # ---------------- LN2 + h2 ----------------
    p_h2 = tc.alloc_tile_pool(name="h2", bufs=KT)
    p_ln2 = tc.alloc_tile_pool(name="ln2", bufs=1)
    rstd2_rep, musc2_rep = ln_chain([ms2], [ss2], NQ, p_ln2, ps_ln2b, ps_st2, "ln2")

    p_tmp2 = tc.alloc_tile_pool(name="tmp2", bufs=3)
    h2t = []
    for k in range(KT):
        tmp = p_tmp2.tile([128, NQ], F32, tag="tmp2")
        nc.vector.tensor_mul(tmp, xb2t[k], rstd2_rep)
        h = p_h2.tile([128, NQ], BF16, tag="h2")
        nc.vector.tensor_sub(h, tmp, musc2_rep)
        h2t.append(h)
    p_tmp2.release()
    ps_ln2b.release()
    ps_st2.release()

    # ---------------- MLP: fc1 bf16, fc2 fp8 DR ----------------
    p_g = tc.alloc_tile_pool(name="g", bufs=JT2)
    p_y = tc.alloc_tile_pool(name="y", bufs=3)
    ps_m = tc.alloc_tile_pool(name="ps_m", bufs=8, space="PSUM")

    g_pair = [p_g.tile([128, 2, NQ], FP8, tag="g", name=f"gp{j}")
              for j in range(JT2)]

    def gelu_out(m, ps):
        if has_bias["fc1"]:
            nc.tensor.matmul(ps, bias_sb[:, ds(b1_of + m * 128, 128)],
                             ones_r16, start=False, stop=True)
        nc.scalar.activation(g_pair[m // 2][:, m % 2, :], ps, gelu_func)

    # first 8 fc1 output tiles k-outer: overlaps the h2 normalize
    ps8 = [ps_m.tile([128, 512], F32, tag="ps_m", name=f"ps8_{m}")
           for m in range(8)]
    for k in range(KT):
        for m in range(8):
            nc.tensor.matmul(
                ps8[m], w1_groups[m // 4][:, ds((m % 4) * C + k * 128, 128)],
                h2t[k],
                start=(k == 0), stop=(k == KT - 1 and not has_bias["fc1"]))
    for m in range(8):
        gelu_out(m, ps8[m])
    for m in range(8, MT1):
        gi = m // 4
        for la in (gi, gi + 1):  # ensure current + one-group lookahead
            if la < MT1 // 4 and la not in w1_groups:
                w1_groups[la] = p_w1.tile([128, 4 * C], BF16, tag="w1",
                                          name=f"w1g{la}")
                nc.sync.dma_start(w1_groups[la], w1g[la, :, :])
        ps = ps_m.tile([128, 512], F32, tag="ps_m")
        for k in range(KT):
            nc.tensor.matmul(
                ps, w1_groups[gi][:, ds((m % 4) * C + k * 128, 128)], h2t[k],
                start=(k == 0), stop=(k == KT - 1 and not has_bias["fc1"]))
        gelu_out(m, ps)

    for m in range(KT):
        la = m + 2
        if la < KT and la not in w2_tiles:
            w2_tiles[la] = p_w2.tile([128, HID], FP8, tag="w2",
                                     name=f"w2p{la}")
            nc.sync.dma_start(w2_tiles[la], w2DR[la, :, :])
        w2t = w2_tiles[m]
        ps = ps_m.tile([128, 512], F32, tag="ps_m")
        for j in range(JT2):
            nc.tensor.matmul(
                ps, w2t[:, ds(j * 256, 256)].rearrange(
                    "p (i f) -> p i f", i=2),
                g_pair[j],
                start=(j == 0), stop=(j == JT2 - 1 and not has_bias["fc2"]),
                perf_mode=DR)
        if has_bias["fc2"]:
            nc.tensor.matmul(ps, bias_sb[:, ds(b2_of + m * 128, 128)],
                             ones_r16, start=False, stop=True)
        y = p_y.tile([128, NQ], F32, tag="y")
        nc.vector.scalar_tensor_tensor(
            y, ps, Y2M, x2[m], ALU.mult, ALU.add)
        nc.sync.dma_start(yT[ds(m * 128, 128), :], y)

    for p in (p_y, p_g, p_ln2, p_h2, p_xb2, p_w1, p_w2, p_x2):
        p.release()
    ps_m.release()
    pers.release()


# --------------------------------------------------------------------------
# Host side
# --------------------------------------------------------------------------
def _pair_m(w, mtiles):
    """fp8 DoubleRow stationary layout for out^T = w^T @ act:
    out[t, p, j*256 + i*128 + f] = w[(2j+i)*128 + p, t*128 + f]."""
    kin = w.shape[0]
    jt = kin // 256
    a = w.reshape(jt, 2, 128, mtiles, 128)     # [j, i, p, t, f]
    return np.ascontiguousarray(
        a.transpose(3, 2, 0, 1, 4).reshape(mtiles, 128, jt * 256))


def _pair_r(w):
    """fp8 DoubleRow moving layout, packed: [128, JT*2*F]:
    out[p, j*2F + i*F + f] = w[(2j+i)*128+p, f]."""
    kin, f = w.shape
    jt = kin // 256
    a = w.reshape(jt, 2, 128, f)               # [j, i, p, f]
    return np.ascontiguousarray(a.transpose(2, 0, 1, 3).reshape(128, jt * 2 * f))


def _m_slice(w, mtiles):
    """[K_in, M_out] -> [mtiles, 128, K_in] with free dim k-major."""
    kin = w.shape[0]
    kt = kin // 128
    a = w.reshape(kt, 128, mtiles, 128)        # [k, i, m, j]
    return np.ascontiguousarray(a.transpose(2, 1, 0, 3).reshape(mtiles, 128, kin))


def _feat_pack(xt, n):
    """[C, n] -> [128, KT*n]: out[p, k*n + c] = xt[k*128 + p, c]."""
    return np.ascontiguousarray(
        xt.reshape(KT, 128, n).transpose(1, 0, 2).reshape(128, KT * n))


def _prep(inputs):
    f32 = np.float32
    x = np.asarray(inputs["x"], f32)
    ln1_g = np.asarray(inputs["ln1_g"], f32)
    ln1_b = np.asarray(inputs["ln1_b"], f32)
    ln2_g = np.asarray(inputs["ln2_g"], f32)
    ln2_b = np.asarray(inputs["ln2_b"], f32)
    w_qkv = np.asarray(inputs["w_qkv"], f32)
    w_proj = np.asarray(inputs["w_proj"], f32)
    w_fc1 = np.asarray(inputs["w_fc1"], f32)
    w_fc2 = np.asarray(inputs["w_fc2"], f32)

    wqkv_e = ln1_g[:, None] * w_qkv
    bqkv_e = ln1_b @ w_qkv + np.asarray(inputs["b_qkv"], f32)
    wfc1_e = ln2_g[:, None] * w_fc1
    bfc1_e = ln2_b @ w_fc1 + np.asarray(inputs["b_fc1"], f32)
    b_proj = np.asarray(inputs["b_proj"], f32)
    b_fc2 = np.asarray(inputs["b_fc2"], f32)

    bf = ml_dtypes.bfloat16
    f8 = ml_dtypes.float8_e4m3
    wq, wk, wvv = wqkv_e[:, :C], wqkv_e[:, C:2 * C], wqkv_e[:, 2 * C:]

    def q8(a):
        return np.clip(a * WSCALE, -240, 240).astype(f8)

    w1s = _m_slice(wfc1_e, MT1)                 # [32, 128, C]
    sel2 = np.zeros((2, 128), ml_dtypes.bfloat16)
    sel2[0, :64] = 1.0
    sel2[1, 64:] = 1.0
    shared = {
        "wkq8": np.concatenate(
            [q8(_pair_m(wk, KT)), q8(_pair_m(wq, KT))], axis=2),
        "wv8": q8(_pair_r(wvv)),
        "wpDR": q8(_pair_m(w_proj, KT)),
        "w1g": np.ascontiguousarray(
            w1s.reshape(MT1 // 4, 4, 128, C).transpose(0, 2, 1, 3)
            .reshape(MT1 // 4, 128, 4 * C)).astype(bf),
        "w2DR": q8(_pair_m(w_fc2, KT)),
        "sel2in": sel2,
        # qkv biases ride the fp8-scaled psum; proj/fc2 biases likewise
        "b_all": np.concatenate(
            [bqkv_e * WSCALE, b_proj * (OSC * WSCALE), bfc1_e,
             b_fc2 * WSCALE])[None, :].astype(bf),
    }
    has_bias = {
        "qk": bool(np.any(bqkv_e[:2 * C])),
        "v": bool(np.any(bqkv_e[2 * C:])),
        "proj": bool(np.any(b_proj)),
        "fc1": bool(np.any(bfc1_e)),
        "fc2": bool(np.any(b_fc2)),
    }

    in_maps = []
    for c in range(8):
        b, half = c // 2, c % 2
        xb = x[b]
        if half:
            xb = np.concatenate([xb[NQ:], xb[:NQ]], axis=0)
        xt = np.ascontiguousarray(xb.T)
        m = {"xTp": _feat_pack(xt[:, :NQ], NQ),
             "xbp": _feat_pack(xt, NTOK).astype(bf),
             **shared}
        in_maps.append(m)
    return in_maps, has_bias


def kernel(**inputs):
    in_maps, has_bias = _prep(inputs)
    key = tuple(sorted(has_bias.items()))
    if key not in _cache:
        nc = build_program(has_bias)
        _split_wide_waits(nc, 1)
        _cache[key] = nc
    nc = _cache[key]

    res = bass_utils.run_bass_kernel_spmd(
        nc, in_maps, core_ids=list(range(8)), trace=False)

    x = np.asarray(inputs["x"])
    out = np.empty((4, NTOK, C), dtype=np.float32)
    for c in range(8):
        b, half = c // 2, c % 2
        out[b, half * NQ:(half + 1) * NQ, :] = res.results[c]["yT"].T
    return out.astype(x.dtype, copy=False)


# revision 18
# speedup vs baseline: 1.1346x; 1.0191x over previous
"""Trainium2 Bass kernel for a pre-norm transformer block (nn_Block).

Math (per batch b of x [4, 1024, 1024]):
    h  = LN(x) ; qkv = h @ w_qkv + b_qkv ; attention (16 heads, dh=64)
    x  = x + (attn_out @ w_proj + b_proj)
    h  = LN(x) ; x = x + gelu(h @ w_fc1 + b_fc1) @ w_fc2 + b_fc2

Sharding: communication-free hybrid over 8 cores. Core c handles batch
b = c // 2 and query-token half c % 2. Each core computes K and V for its
batch's full 1024 tokens and everything else for its own 512 queries.

Precision: K/Q/V, S^T, exp(P), PV, proj and fc2 run in fp8-e4m3 with
DoubleRow (2 contraction blocks per matmul, ~2x); fc1 stays bf16 (fp8 on
both fc matmuls measured 2.4e-2 end-to-end, over the 2e-2 gate; fc2-only
measured ~1.7e-2).

vs the previous 344us version (trace-driven):
  - softmax exp was the serializer (64 ACT EXPs ~1.15us each gating the
    S^T psum ping-pong; PE starved in 1-1.7us gaps and HAM re-throttled
    it to 1.2GHz for ~60us). Exps now split: ACT keeps half, the vector
    engine computes the rest as Schraudolph exp (i32 = A*S + B via
    tensor_scalar convert, bitcast back to f32, copy to fp8; rms err
    ~1.8% vs e4m3's 3.6% quantization - end-to-end delta +1e-5).
  - LN resolve used a DRAM-bounce row broadcast (8.5us dead latency) and
    mixed DVE/GpSimd normalize (SBUF port collisions tripled op time).
    Rows now broadcast via a rank-1 PE matmul (ones[1,128] x row[1,512]
    -> psum) and normalize runs DVE-only on bf16.
  - PV softmax denominators ride a 1/16-column through the fp8 PV psum;
    ACT's Exp(-Ln(den/16)) = 16/den is exactly the fp8 O scale; the
    per-head-pair reciprocal rows broadcast through the same PE trick.
  - proj/fc2 weights host-packed into DoubleRow pair layout; O and gelu
    outputs written as fp8 pair tiles; psum scales folded into the
    residual-add (scalar_tensor_tensor) evicts.
  - K/Q production all happens inside the V phase (frees 2 psum banks ->
    S^T runs a 2x[128,1024] ping-pong + 3 PV banks + 1 broadcast bank).
  - dummy matmuls chained on the LN row chain keep the PE HAM warm
    across the two LN windows.
"""

import os
import sys

import numpy as np

try:
    import concourse.bass as bass
except ImportError:  # pragma: no cover
    for _p in ("/opt/trn_rl_repo", "/root/.axon_site/_ro/trn_rl_repo"):
        if os.path.isdir(_p) and _p not in sys.path:
            sys.path.insert(0, _p)
    import concourse.bass as bass

import ml_dtypes
import concourse.tile as tile
import concourse.mybir as mybir
from concourse import bass_utils
from concourse.bass import ds

F32 = mybir.dt.float32
BF16 = mybir.dt.bfloat16
FP8 = mybir.dt.float8e4
FP8E5 = mybir.dt.float8e5
I32 = mybir.dt.int32
AF = mybir.ActivationFunctionType
ALU = mybir.AluOpType
DR = mybir.MatmulPerfMode.DoubleRow

C = 1024          # model dim
H = 16            # heads
DH = 64           # head dim
NTOK = 1024       # tokens per batch (keys/values)
NQ = 512          # query tokens per core
KT = C // 128     # 8 feature tiles
JT = KT // 2      # 4 feature-pair tiles (DoubleRow)
HID = 4096
MT1 = HID // 128  # 32 fc1 output tiles
JT2 = HID // 256  # 16 fc2 contraction pairs
EPS = 1e-5
WSCALE = 2048.0   # pow2 scale folded into fp8 weights
QSM = 1.0 / (WSCALE * float(DH) ** 0.5)   # Q psum -> fp8 cast scale
KSM = 1.0 / WSCALE                        # K/V psum -> fp8 cast scale
OSC = 16.0                                # fp8 O scale (from 1/16 ones col)
PSM = 1.0 / (OSC * WSCALE)                # proj psum -> f32 scale
Y2M = 1.0 / WSCALE                        # fc2 psum -> f32 scale

# Schraudolph exp emitting fp8-e5m2 bits in the int32's top byte:
# j = 8*(2^23/ln2)*x + (8*(15*2^23 - 486411) + 2^23); P = byte3(j)
EXP_A = float(8 * 2 ** 23 / np.log(2.0))
EXP_B = float(8 * (15 * 2 ** 23 - 486411) + 2 ** 23)


def exp_eng(t, r, h2):
    # 'A' ACT table exp (fp8e4 P); 'D' DVE schraudolph (e5m2 P view)
    if h2 == 0 or (r == 0 and t % 2 == 0):
        return 'A'
    return 'D'

_cache = {}


def _split_wide_waits(nc, max_waits=1):
    """Walrus on this image rejects instructions carrying more than one
    semaphore wait; split the excess onto same-engine NOPs."""
    ctr = 0
    for f in nc.m.functions:
        for b in f.blocks:
            out, changed = [], False
            for inst in b.instructions:
                si = getattr(inst, "sync_info", None)
                if si is not None and si.on_wait and len(si.on_wait) > max_waits:
                    waits = list(si.on_wait)
                    extra, keep = waits[:-max_waits], waits[-max_waits:]
                    for gs in range(0, len(extra), max_waits):
                        ctr += 1
                        nop = mybir.InstNoOp(
                            name=f"waitsplit-{ctr}", ins=[], outs=[])
                        nop.engine = inst.engine
                        nop.sync_info = mybir.SyncInfo(
                            on_wait=extra[gs:gs + max_waits], on_update=[])
                        out.append(nop)
                    inst.sync_info = mybir.SyncInfo(
                        on_wait=keep, on_update=list(si.on_update))
                    changed = True
                out.append(inst)
            if changed:
                b.instructions = out


def build_program(has_bias, gelu_func=None):
    nc = bass.Bass()

    xTp = nc.dram_tensor("xTp", [128, KT * NQ], F32, kind="ExternalInput")
    xbp = nc.dram_tensor("xbp", [128, KT * NTOK], BF16, kind="ExternalInput")
    wkq8 = nc.dram_tensor("wkq8", [KT, 128, 2 * C], FP8, kind="ExternalInput")
    wv8 = nc.dram_tensor("wv8", [128, JT * 2 * C], FP8, kind="ExternalInput")
    wpDR = nc.dram_tensor("wpDR", [KT, 128, C], FP8, kind="ExternalInput")
    sel2in = nc.dram_tensor("sel2in", [2, 128], BF16, kind="ExternalInput")
    w1g = nc.dram_tensor("w1g", [MT1 // 4, 128, 4 * C], BF16,
                         kind="ExternalInput")
    w2DR = nc.dram_tensor("w2DR", [KT, 128, HID], FP8, kind="ExternalInput")
    b_all = nc.dram_tensor("b_all", [1, 3 * C + C + HID + C], BF16,
                           kind="ExternalInput")
    yT = nc.dram_tensor("yT", [C, NQ], F32, kind="ExternalOutput")

    with tile.TileContext(nc) as tc:
        _emit(nc, tc, xTp, xbp, wkq8, wv8, wpDR, w1g, w2DR, b_all,
              sel2in, yT, has_bias, gelu_func or AF.Gelu)
    return nc


def _emit(nc, tc, xTp, xbp, wkq8, wv8, wpDR, w1g, w2DR, b_all,
          sel2in, yT, has_bias, gelu_func):
    pers = tc.alloc_tile_pool(name="pers", bufs=1)
    ones_c = pers.tile([128, 1], BF16, tag="ones_c")      # stats lhsT
    nc.vector.memset(ones_c, 1.0)
    ones_r16 = pers.tile([1, NQ], BF16, tag="ones_r16")   # bias rank-1 rhs
    nc.vector.memset(ones_r16, 1.0)
    ones_tok16 = pers.tile([1, 128], BF16, tag="ones_tok16")  # v-bias lhsT
    nc.vector.memset(ones_tok16, 1.0)
    ones_b = pers.tile([1, 128], BF16, tag="ones_b")      # broadcast lhsT
    nc.vector.memset(ones_b, 1.0)
    sel2 = pers.tile([2, 128], BF16, tag="sel2")          # 2-head bcast lhsT
    nc.sync.dma_start(sel2, sel2in[:, :])
    eps_t = pers.tile([128, 1], F32, tag="eps_t")
    nc.vector.memset(eps_t, EPS)
    junk_l = pers.tile([1, 128], BF16, tag="junk_l")      # keep-warm lhsT
    nc.vector.memset(junk_l, 0.0)

    any_bias = any(has_bias.values())
    if any_bias:
        bias_sb = pers.tile([1, 3 * C + C + HID + C], BF16, tag="bias_sb")
        nc.sync.dma_start(bias_sb, b_all[:])
        bq_of, bk_of, bv_of = 0, C, 2 * C
        bp_of, b1_of, b2_of = 3 * C, 4 * C, 4 * C + HID

    # pools ordered by lifetime (latest-dying first): releases are a
    # strict LIFO stack per memory space
    p_x2 = tc.alloc_tile_pool(name="x2", bufs=KT)
    p_w2 = tc.alloc_tile_pool(name="w2", bufs=3)
    p_w1 = tc.alloc_tile_pool(name="w1", bufs=3)
    p_xb2 = tc.alloc_tile_pool(name="xb2", bufs=KT)
    p_xT = tc.alloc_tile_pool(name="xT", bufs=1)
    p_wp = tc.alloc_tile_pool(name="wp", bufs=1)
    p_O = tc.alloc_tile_pool(name="O", bufs=JT)
    p_sq2 = tc.alloc_tile_pool(name="sq2", bufs=3)
    p_V = tc.alloc_tile_pool(name="V", bufs=JT)
    p_K = tc.alloc_tile_pool(name="K", bufs=KT)
    p_Q = tc.alloc_tile_pool(name="Q", bufs=KT)
    p_h1 = tc.alloc_tile_pool(name="h1", bufs=JT)
    p_wv = tc.alloc_tile_pool(name="wv", bufs=1)
    p_wkq = tc.alloc_tile_pool(name="wkq", bufs=4)
    p_xb = tc.alloc_tile_pool(name="xb", bufs=1)
    p_ln1 = tc.alloc_tile_pool(name="ln1", bufs=1)
    p_sq = tc.alloc_tile_pool(name="sq", bufs=3)
    ps_stat = tc.alloc_tile_pool(name="ps_stat", bufs=1, space="PSUM")
    ps_lnb = tc.alloc_tile_pool(name="ps_lnb", bufs=2, space="PSUM")

    warm_n = [0]

    def keep_warm(warm_ps, dep_row, n=3):
        """Chain n dummy matmuls on dep_row (a [1,>=1] tile AP) so the PE
        HAM sees activity while the LN row chain resolves."""
        s = warm_n[0] * 3 % 96
        warm_n[0] += 1
        nc.vector.tensor_copy(junk_l[:, ds(s, 1)], dep_row[:, ds(0, 1)])
        for i in range(n):
            nc.tensor.matmul(warm_ps, junk_l, ones_r16, start=True, stop=True)

    def ln_chain(ms, ss, N, pool, pspool, warmpool, nm):
        """From per-chunk sum/sumsq psum rows produce [128, N] bf16
        rstd_rep and (mu*rstd)_rep via rank-1 PE broadcasts."""
        nch = N // 512
        warm_ps = warmpool.tile([128, NQ], F32, tag="warm", name=f"warm{nm}")
        rrep = pool.tile([128, N], BF16, tag=f"rrep_{nm}", name=f"rrep_{nm}")
        mrep = pool.tile([128, N], BF16, tag=f"mrep_{nm}", name=f"mrep_{nm}")
        for n in range(nch):
            tn = pool.tile([1, 512], F32, tag=f"tn_{nm}", name=f"tn_{nm}{n}")
            nc.scalar.activation(tn, ms[n], AF.Square)
            keep_warm(warm_ps, tn)
            vn = pool.tile([1, 512], F32, tag=f"vn_{nm}", name=f"vn_{nm}{n}")
            nc.vector.scalar_tensor_tensor(
                vn, tn, 1.0 / C, ss[n], ALU.mult, ALU.subtract)
            lnv = pool.tile([1, 512], F32, tag=f"lnv_{nm}",
                            name=f"lnv_{nm}{n}")
            nc.scalar.activation(lnv, vn, AF.Ln, bias=eps_t[ds(0, 1), :],
                                 scale=-1.0 / C)
            keep_warm(warm_ps, lnv)
            rsr = pool.tile([1, 512], BF16, tag=f"rsr_{nm}",
                            name=f"rsr_{nm}{n}")
            nc.scalar.activation(rsr, lnv, AF.Exp, scale=-0.5)
            msr = pool.tile([1, 512], BF16, tag=f"msr_{nm}",
                            name=f"msr_{nm}{n}")
            nc.vector.scalar_tensor_tensor(
                msr, ms[n], 1.0 / C, rsr, ALU.mult, ALU.mult)
            keep_warm(warm_ps, msr)
            rst_ps = pspool.tile([128, 512], F32, tag="lnb",
                                 name=f"rst_ps{nm}{n}")
            nc.tensor.matmul(rst_ps, ones_b, rsr, start=True, stop=True)
            nc.vector.tensor_copy(rrep[:, ds(n * 512, 512)], rst_ps)
            mus_ps = pspool.tile([128, 512], F32, tag="lnb",
                                 name=f"mus_ps{nm}{n}")
            nc.tensor.matmul(mus_ps, ones_b, msr, start=True, stop=True)
            nc.vector.tensor_copy(mrep[:, ds(n * 512, 512)], mus_ps)
        return rrep, mrep

    # ---- bulk loads: one big DMA each, SBUF views per tile ----
    xb_all = p_xb.tile([128, KT * NTOK], BF16, tag="xb")
    for q in range(4):
        nc.sync.dma_start(xb_all[:, ds(q * 2 * NTOK, 2 * NTOK)],
                          xbp[:, ds(q * 2 * NTOK, 2 * NTOK)])
    xbt = [xb_all[:, ds(k * NTOK, NTOK)] for k in range(KT)]

    wv_all = p_wv.tile([128, JT * 2 * C], FP8, tag="wv")
    nc.gpsimd.dma_start(wv_all, wv8[:, :])
    wv = [wv_all[:, ds(j * 2 * C, 2 * C)] for j in range(JT)]

    # prefetch K/Q weights for the first head pair only
    wkq_tiles = {}
    for t0 in range(2):
        w = p_wkq.tile([128, 2 * C], FP8, tag="wkq", name=f"wkq{t0}")
        nc.gpsimd.dma_start(w, wkq8[t0, :, :])
        wkq_tiles[t0] = w

    # V2[r]: pair layout [128 tok, i(2), H, 65] fp8; [.., 64] is the 1/16 col
    V2 = []
    for r in range(JT):
        vt = p_V.tile([128, 2, H, 65], FP8, tag="V", name=f"V{r}")
        nc.vector.memset(vt[:, :, :, ds(64, 1)], 1.0 / OSC)
        V2.append(vt)

    # ---- LN1 stats ----
    ms = [ps_stat.tile([1, 512], F32, tag=f"ms{n}", name=f"ms{n}")
          for n in range(2)]
    ss = [ps_stat.tile([1, 512], F32, tag=f"ss{n}", name=f"ss{n}")
          for n in range(2)]
    for k in range(KT):
        sq = p_sq.tile([128, NTOK], BF16, tag="sq")
        if k % 4 == 2:
            nc.scalar.activation(sq, xbt[k], AF.Square)
        else:
            nc.vector.tensor_mul(sq, xbt[k], xbt[k])
        for n in range(2):
            nc.tensor.matmul(ms[n], ones_c, xbt[k][:, ds(n * 512, 512)],
                             start=(k == 0), stop=(k == KT - 1))
            nc.tensor.matmul(ss[n], ones_c, sq[:, ds(n * 512, 512)],
                             start=(k == 0), stop=(k == KT - 1))
    p_sq.release()

    rstd_rep, musc_rep = ln_chain(ms, ss, NTOK, p_ln1, ps_lnb, ps_stat, "ln1")

    # h1p[j]: fp8 pair tile [128, 2, NTOK]; halves are feature blocks 2j,2j+1
    p_tmp = tc.alloc_tile_pool(name="tmp", bufs=3)
    h1 = [p_h1.tile([128, 2, NTOK], FP8, tag="h1", name=f"h1p{j}")
          for j in range(JT)]
    for k in range(KT):
        tmp = p_tmp.tile([128, NTOK], F32, tag="tmp")
        nc.vector.tensor_mul(tmp, xbt[k], rstd_rep)
        nc.vector.tensor_sub(h1[k // 2][:, k % 2, :], tmp, musc_rep)
    p_tmp.release()

    # bulk proj-phase loads anchored on the last h1 tile so the bursts land
    # during the matmul-heavy V/attention phases
    xt = p_xT.tile([128, KT * NQ], F32, tag="xT")
    wp_all = p_wp.tile([128, KT * C], FP8, tag="wp")
    nc.vector.tensor_copy(xt[ds(0, 1), ds(0, 1)], h1[3][ds(0, 1), 1, ds(0, 1)])
    nc.gpsimd.dma_start(xt, xTp[:, :])
    nc.vector.tensor_copy(wp_all[ds(0, 1), ds(0, 1)],
                          h1[3][ds(0, 1), 1, ds(0, 1)])
    for m in range(KT):
        nc.gpsimd.dma_start(wp_all[:, ds(m * C, C)], wpDR[m, :, :])
    p_ln1.release()
    p_xb.release()
    ps_lnb.release()
    ps_stat.release()

    # ---------------- V + K + Q (token-major, DoubleRow) ----------------
    ps_kq = tc.alloc_tile_pool(name="ps_kq", bufs=2, space="PSUM")
    ps_v = tc.alloc_tile_pool(name="ps_v", bufs=6, space="PSUM")

    K_sb, Q_sb, P_sb, O_pair = [], [], {}, []

    def emit_kq(t):
        wt = wkq_tiles.pop(t)
        wkt, wqt = wt[:, ds(0, C)], wt[:, ds(C, C)]
        if t + 2 < KT:  # keep two pairs in flight
            nw = p_wkq.tile([128, 2 * C], FP8, tag="wkq", name=f"wkq{t+2}")
            nc.gpsimd.dma_start(nw, wkq8[t + 2, :, :])
            wkq_tiles[t + 2] = nw
        kt_sb = p_K.tile([128, NTOK], FP8, tag="K")
        wkp = wkt.rearrange("p (j i f) -> p j i f", j=JT, i=2)
        for n in range(2):
            ps = ps_kq.tile([128, 512], F32, tag="ps_kq")
            for j in range(JT):
                nc.tensor.matmul(
                    ps, wkp[:, j],
                    h1[j][:, :, ds(n * 512, 512)],
                    start=(j == 0), stop=(j == JT - 1 and not has_bias["qk"]),
                    perf_mode=DR)
            if has_bias["qk"]:
                nc.tensor.matmul(
                    ps, bias_sb[:, ds(bk_of + t * 128, 128)], ones_r16,
                    start=False, stop=True)
            nc.scalar.mul(kt_sb[:, ds(n * 512, 512)], ps, KSM)
        K_sb.append(kt_sb)

        qt_sb = p_Q.tile([128, NQ], FP8, tag="Q")
        wqp = wqt.rearrange("p (j i f) -> p j i f", j=JT, i=2)
        ps = ps_kq.tile([128, 512], F32, tag="ps_kq")
        for j in range(JT):
            nc.tensor.matmul(
                ps, wqp[:, j], h1[j][:, :, ds(0, 512)],
                start=(j == 0), stop=(j == JT - 1 and not has_bias["qk"]),
                perf_mode=DR)
        if has_bias["qk"]:
            nc.tensor.matmul(
                ps, bias_sb[:, ds(bq_of + t * 128, 128)], ones_r16,
                start=False, stop=True)
        nc.scalar.mul(qt_sb, ps, QSM)
        Q_sb.append(qt_sb)

    # V in groups of 2 feature-tiles with all 8 K/Q pairs interleaved
    kq_queue = list(range(KT))
    for g0 in range(0, KT, 2):
        ts_ = range(g0, min(g0 + 2, KT))
        psv = {(t, n): ps_v.tile([128, 512], F32, tag="ps_v",
                                 name=f"psv{t}_{n}")
               for t in ts_ for n in range(2)}
        for j in range(JT):
            for t in ts_:
                for n in range(2):
                    nc.tensor.matmul(
                        psv[(t, n)], h1[j][:, :, ds(t * 128, 128)],
                        wv[j].rearrange("p (i f) -> p i f", i=2)[
                            :, :, ds(n * 512, 512)],
                        start=(j == 0),
                        stop=(j == JT - 1 and not has_bias["v"]),
                        perf_mode=DR)
        for t in ts_:
            for n in range(2):
                if has_bias["v"]:
                    nc.tensor.matmul(
                        psv[(t, n)], ones_tok16,
                        bias_sb[:, ds(bv_of + n * 512, 512)],
                        start=False, stop=True)
                nc.scalar.mul(
                    V2[t // 2][:, t % 2, ds(n * 8, 8), ds(0, 64)],
                    psv[(t, n)].rearrange("p (h d) -> p h d", d=64), KSM)
        for _ in range(2):
            if kq_queue:
                emit_kq(kq_queue.pop(0))

    ps_v.release()
    ps_kq.release()
    p_wkq.release()
    p_wv.release()
    p_h1.release()

    # ---------------- attention: S^T + softmax + PV ---------------------
    p_P = tc.alloc_tile_pool(name="P", bufs=10)
    p_i32 = tc.alloc_tile_pool(name="i32", bufs=6)
    p_den = tc.alloc_tile_pool(name="den", bufs=3)
    p_rep = tc.alloc_tile_pool(name="rep", bufs=2)
    ps_s = tc.alloc_tile_pool(name="ps_s", bufs=2, space="PSUM")
    ps_pv = tc.alloc_tile_pool(name="ps_pv", bufs=3, space="PSUM")
    ps_rep = tc.alloc_tile_pool(name="ps_rep", bufs=1, space="PSUM")

    for j in range(JT):
        O_pair.append(p_O.tile([128, 2, NQ], FP8, tag="O", name=f"Op{j}"))

    def emit_st(t):
        for r in range(JT):
            pss = {h2: ps_s.tile([128, 2, 512], F32, tag="ps_s",
                                 name=f"pss{t}_{r}_{h2}")
                   for h2 in range(2)}
            for i in range(2):
                m = 2 * r + i
                for h2 in range(2):
                    lo = h2 * 64
                    nc.tensor.matmul(
                        pss[h2][:, i, :],
                        K_sb[t][ds(lo, 64), ds(m * 128, 128)],
                        Q_sb[t][ds(lo, 64), :],
                        start=True, stop=True)
            for h2 in range(2):
                flat_in = pss[h2].rearrange("p i f -> p (i f)")
                if exp_eng(t, r, h2) == 'A':
                    p = p_P.tile([128, 2, 512], FP8, tag="P")
                    nc.scalar.activation(
                        p.rearrange("p i f -> p (i f)"), flat_in, AF.Exp)
                    P_sb[(t, h2, r)] = p
                else:
                    it = p_i32.tile([128, 1024], I32, tag="i32")
                    nc.vector.tensor_scalar(
                        it, flat_in, EXP_A, EXP_B, ALU.mult, ALU.add)
                    # byte 3 of each int32 is the e5m2 P bit pattern
                    P_sb[(t, h2, r)] = it[:, :].bitcast(FP8E5).rearrange(
                        "p (i f b) -> p i f b", i=2, b=4)[:, :, :, ds(3, 1)
                        ].rearrange("p i f b -> p i (f b)")

    def emit_pv(t):
        # PV with the 1/16-column denominator in psum row 64. den rows ->
        # ACT Exp(-Ln) = 16/den -> rank-1 PE broadcast -> GpSimd evicts
        # O pair halves as (psum * rcp_rep) in fp8.
        den2 = p_den.tile([2, 512], BF16, tag="den", name=f"den{t}")
        pvs = {}
        for h2 in range(2):
            head = 2 * t + h2
            ps = ps_pv.tile([65, 512], F32, tag="ps_pv", name=f"pspv{t}_{h2}")
            for r in range(JT):
                nc.tensor.matmul(
                    ps, V2[r][:, :, head, :], P_sb[(t, h2, r)],
                    start=(r == 0), stop=(r == JT - 1),
                    perf_mode=DR)
            if h2 == 0:
                nc.vector.tensor_copy(den2[ds(0, 1), :], ps[ds(64, 1), :])
            else:
                dtmp = p_den.tile([1, 512], BF16, tag="dtmp",
                                  name=f"dtmp{t}")
                nc.vector.tensor_copy(dtmp, ps[ds(64, 1), :])
                nc.sync.dma_start(den2[ds(1, 1), :], dtmp)
            pvs[h2] = ps
        lnr = p_den.tile([2, 512], F32, tag="lnr", name=f"lnr{t}")
        nc.scalar.activation(lnr, den2, AF.Ln)
        rcp2 = p_den.tile([2, 512], BF16, tag="rcp", name=f"rcp{t}")
        nc.scalar.activation(rcp2, lnr, AF.Exp, scale=-1.0)
        rps = ps_rep.tile([128, 512], F32, tag="ps_rep", name=f"rps{t}")
        nc.tensor.matmul(rps, sel2, rcp2, start=True, stop=True)
        rep = p_rep.tile([128, 512], BF16, tag="rep")
        nc.vector.tensor_copy(rep, rps)
        # h2=0: DVE multiplies straight from psum; h2=1: DVE evicts to
        # bf16 and GpSimd (no PSUM access) does the multiply in SBUF
        nc.vector.tensor_mul(
            O_pair[t // 2][ds(0, 64), t % 2, :],
            pvs[0][ds(0, 64), :], rep[ds(0, 64), :])
        otb = p_rep.tile([128, 512], BF16, tag="otb", name=f"otb{t}")
        nc.vector.tensor_copy(otb[ds(64, 64), :], pvs[1][ds(0, 64), :])
        nc.gpsimd.tensor_mul(
            O_pair[t // 2][ds(64, 64), t % 2, :],
            otb[ds(64, 64), :], rep[ds(64, 64), :])

    # keep-warm burst bridging the V->attention transition (the HAM
    # re-throttles on a >3.4us PE-idle window and the gapped attention
    # stream cannot re-warm it)
    wps = ps_rep.tile([128, 512], F32, tag="ps_rep", name="warm_v2s")
    nc.vector.tensor_copy(junk_l[:, ds(96, 1)], Q_sb[7][ds(0, 1), ds(0, 1)])
    for _ in range(16):
        nc.tensor.matmul(wps, junk_l, ones_r16, start=True, stop=True)

    w1_groups, w2_tiles = {}, {}
    for t in range(KT):
        emit_st(t)
        if t >= 1:
            emit_pv(t - 1)
        if 3 <= t <= 7:
            # fc1/fc2 weights: anchored so each burst lands inside the
            # attention matmul phase
            anchor = O_pair[(t - 3) // 2][ds(0, 1), (t - 3) % 2, ds(0, 1)]
            i = t - 3
            if i < 3:
                w1_groups[i] = p_w1.tile([128, 4 * C], BF16, tag="w1",
                                         name=f"w1g{i}")
                nc.vector.tensor_copy(
                    w1_groups[i][ds(0, 1), ds(0, 1)], anchor)
                nc.sync.dma_start(w1_groups[i], w1g[i, :, :])
            else:
                w2_tiles[i - 3] = p_w2.tile([128, HID], FP8, tag="w2",
                                            name=f"w2p{i-3}")
                nc.vector.tensor_copy(
                    w2_tiles[i - 3][ds(0, 1), ds(0, 1)], anchor)
                nc.sync.dma_start(w2_tiles[i - 3], w2DR[i - 3, :, :])
    wps2 = ps_rep.tile([128, 512], F32, tag="ps_rep", name="warm_pv")
    nc.vector.tensor_copy(junk_l[:, ds(97, 1)],
                          P_sb[(KT - 1, 0, JT - 1)][ds(0, 1), 0, ds(0, 1)])
    for _ in range(10):
        nc.tensor.matmul(wps2, junk_l, ones_r16, start=True, stop=True)
    emit_pv(KT - 1)
    for p in (p_rep, p_den, p_i32, p_P, p_Q, p_K, p_V):
        p.release()
    for p in (ps_rep, ps_pv, ps_s):
        p.release()

    # ---------------- proj (fp8 DR) + residual + LN2 stats ----------------
    ps_st2 = tc.alloc_tile_pool(name="ps_st2", bufs=1, space="PSUM")
    ps_ln2b = tc.alloc_tile_pool(name="ps_ln2b", bufs=2, space="PSUM")
    ps_p = tc.alloc_tile_pool(name="ps_p", bufs=3, space="PSUM")

    ms2 = ps_st2.tile([1, 512], F32, tag="ms2")
    ss2 = ps_st2.tile([1, 512], F32, tag="ss2")
    x2, xb2t = [], []
    for m in range(KT):
        ps = ps_p.tile([128, 512], F32, tag="ps_p")
        wpp_m = wp_all[:, ds(m * C, C)]
        for j in range(JT):
            nc.tensor.matmul(
                ps, wpp_m[:, ds(j * 256, 256)].rearrange(
                    "p (i f) -> p i f", i=2),
                O_pair[j],
                start=(j == 0), stop=(j == JT - 1 and not has_bias["proj"]),
                perf_mode=DR)
        if has_bias["proj"]:
            nc.tensor.matmul(ps, bias_sb[:, ds(bp_of + m * 128, 128)],
                             ones_r16, start=False, stop=True)
        xm = p_x2.tile([128, NQ], F32, tag="x2")
        nc.vector.scalar_tensor_tensor(
            xm, ps, PSM, xt[:, ds(m * NQ, NQ)], ALU.mult, ALU.add)
        x2.append(xm)
        xb2 = p_xb2.tile([128, NQ], BF16, tag="xb2", name=f"xb2_{m}")
        nc.vector.tensor_copy(xb2, xm)
        xb2t.append(xb2)
        sq = p_sq2.tile([128, NQ], BF16, tag="sq2")
        if m % 2 == 1:
            nc.scalar.activation(sq, xm, AF.Square)
        else:
            nc.vector.tensor_mul(sq, xb2, xb2)
        nc.tensor.matmul(ms2, ones_c, xb2,
                         start=(m == 0), stop=(m == KT - 1))
        nc.tensor.matmul(ss2, ones_c, sq,
                         start=(m == 0), stop=(m == KT - 1))

    p_sq2.release()
    for p in (p_O, p_wp, p_xT):
        p.release()
    ps_p.release()

    # ---------------- LN2 + h2 ----------------
    p_h2 = tc.alloc_tile_pool(name="h2", bufs=KT)
    p_ln2 = tc.alloc_tile_pool(name="ln2", bufs=1)
    rstd2_rep, musc2_rep = ln_chain([ms2], [ss2], NQ, p_ln2, ps_ln2b, ps_st2, "ln2")

    p_tmp2 = tc.alloc_tile_pool(name="tmp2", bufs=3)
    h2t = []
    for k in range(KT):
        tmp = p_tmp2.tile([128, NQ], F32, tag="tmp2")
        nc.vector.tensor_mul(tmp, xb2t[k], rstd2_rep)
        h = p_h2.tile([128, NQ], BF16, tag="h2")
        nc.vector.tensor_sub(h, tmp, musc2_rep)
        h2t.append(h)
    p_tmp2.release()
    ps_ln2b.release()
    ps_st2.release()

    # ---------------- MLP: fc1 bf16, fc2 fp8 DR ----------------
    p_g = tc.alloc_tile_pool(name="g", bufs=JT2)
    p_y = tc.alloc_tile_pool(name="y", bufs=3)
    ps_m = tc.alloc_tile_pool(name="ps_m", bufs=8, space="PSUM")

    g_pair = [p_g.tile([128, 2, NQ], FP8, tag="g", name=f"gp{j}")
              for j in range(JT2)]

    def gelu_out(m, ps):
        if has_bias["fc1"]:
            nc.tensor.matmul(ps, bias_sb[:, ds(b1_of + m * 128, 128)],
                             ones_r16, start=False, stop=True)
        nc.scalar.activation(g_pair[m // 2][:, m % 2, :], ps, gelu_func)

    # first 8 fc1 output tiles k-outer: overlaps the h2 normalize
    ps8 = [ps_m.tile([128, 512], F32, tag="ps_m", name=f"ps8_{m}")
           for m in range(8)]
    for k in range(KT):
        for m in range(8):
            nc.tensor.matmul(
                ps8[m], w1_groups[m // 4][:, ds((m % 4) * C + k * 128, 128)],
                h2t[k],
                start=(k == 0), stop=(k == KT - 1 and not has_bias["fc1"]))
    for m in range(8):
        gelu_out(m, ps8[m])
    for m in range(8, MT1):
        gi = m // 4
        for la in (gi, gi + 1):  # ensure current + one-group lookahead
            if la < MT1 // 4 and la not in w1_groups:
                w1_groups[la] = p_w1.tile([128, 4 * C], BF16, tag="w1",
                                          name=f"w1g{la}")
                nc.sync.dma_start(w1_groups[la], w1g[la, :, :])
        ps = ps_m.tile([128, 512], F32, tag="ps_m")
        for k in range(KT):
            nc.tensor.matmul(
                ps, w1_groups[gi][:, ds((m % 4) * C + k * 128, 128)], h2t[k],
                start=(k == 0), stop=(k == KT - 1 and not has_bias["fc1"]))
        gelu_out(m, ps)

    for m in range(KT):
        la = m + 2
        if la < KT and la not in w2_tiles:
            w2_tiles[la] = p_w2.tile([128, HID], FP8, tag="w2",
                                     name=f"w2p{la}")
            nc.sync.dma_start(w2_tiles[la], w2DR[la, :, :])
        w2t = w2_tiles[m]
        ps = ps_m.tile([128, 512], F32, tag="ps_m")
        for j in range(JT2):
            nc.tensor.matmul(
                ps, w2t[:, ds(j * 256, 256)].rearrange(
                    "p (i f) -> p i f", i=2),
                g_pair[j],
                start=(j == 0), stop=(j == JT2 - 1 and not has_bias["fc2"]),
                perf_mode=DR)
        if has_bias["fc2"]:
            nc.tensor.matmul(ps, bias_sb[:, ds(b2_of + m * 128, 128)],
                             ones_r16, start=False, stop=True)
        y = p_y.tile([128, NQ], F32, tag="y")
        nc.vector.scalar_tensor_tensor(
            y, ps, Y2M, x2[m], ALU.mult, ALU.add)
        nc.sync.dma_start(yT[ds(m * 128, 128), :], y)

    for p in (p_y, p_g, p_ln2, p_h2, p_xb2, p_w1, p_w2, p_x2):
        p.release()
    ps_m.release()
    pers.release()


# --------------------------------------------------------------------------
# Host side
# --------------------------------------------------------------------------
def _pair_m(w, mtiles):
    """fp8 DoubleRow stationary layout for out^T = w^T @ act:
    out[t, p, j*256 + i*128 + f] = w[(2j+i)*128 + p, t*128 + f]."""
    kin = w.shape[0]
    jt = kin // 256
    a = w.reshape(jt, 2, 128, mtiles, 128)     # [j, i, p, t, f]
    return np.ascontiguousarray(
        a.transpose(3, 2, 0, 1, 4).reshape(mtiles, 128, jt * 256))


def _pair_r(w):
    """fp8 DoubleRow moving layout, packed: [128, JT*2*F]:
    out[p, j*2F + i*F + f] = w[(2j+i)*128+p, f]."""
    kin, f = w.shape
    jt = kin // 256
    a = w.reshape(jt, 2, 128, f)               # [j, i, p, f]
    return np.ascontiguousarray(a.transpose(2, 0, 1, 3).reshape(128, jt * 2 * f))


def _m_slice(w, mtiles):
    """[K_in, M_out] -> [mtiles, 128, K_in] with free dim k-major."""
    kin = w.shape[0]
    kt = kin // 128
    a = w.reshape(kt, 128, mtiles, 128)        # [k, i, m, j]
    return np.ascontiguousarray(a.transpose(2, 1, 0, 3).reshape(mtiles, 128, kin))


def _feat_pack(xt, n):
    """[C, n] -> [128, KT*n]: out[p, k*n + c] = xt[k*128 + p, c]."""
    return np.ascontiguousarray(
        xt.reshape(KT, 128, n).transpose(1, 0, 2).reshape(128, KT * n))


def _prep(inputs):
    f32 = np.float32
    x = np.asarray(inputs["x"], f32)
    ln1_g = np.asarray(inputs["ln1_g"], f32)
    ln1_b = np.asarray(inputs["ln1_b"], f32)
    ln2_g = np.asarray(inputs["ln2_g"], f32)
    ln2_b = np.asarray(inputs["ln2_b"], f32)
    w_qkv = np.asarray(inputs["w_qkv"], f32)
    w_proj = np.asarray(inputs["w_proj"], f32)
    w_fc1 = np.asarray(inputs["w_fc1"], f32)
    w_fc2 = np.asarray(inputs["w_fc2"], f32)

    wqkv_e = ln1_g[:, None] * w_qkv
    bqkv_e = ln1_b @ w_qkv + np.asarray(inputs["b_qkv"], f32)
    wfc1_e = ln2_g[:, None] * w_fc1
    bfc1_e = ln2_b @ w_fc1 + np.asarray(inputs["b_fc1"], f32)
    b_proj = np.asarray(inputs["b_proj"], f32)
    b_fc2 = np.asarray(inputs["b_fc2"], f32)

    bf = ml_dtypes.bfloat16
    f8 = ml_dtypes.float8_e4m3
    wq, wk, wvv = wqkv_e[:, :C], wqkv_e[:, C:2 * C], wqkv_e[:, 2 * C:]

    def q8(a):
        return np.clip(a * WSCALE, -240, 240).astype(f8)

    w1s = _m_slice(wfc1_e, MT1)                 # [32, 128, C]
    sel2 = np.zeros((2, 128), ml_dtypes.bfloat16)
    sel2[0, :64] = 1.0
    sel2[1, 64:] = 1.0
    shared = {
        "wkq8": np.concatenate(
            [q8(_pair_m(wk, KT)), q8(_pair_m(wq, KT))], axis=2),
        "wv8": q8(_pair_r(wvv)),
        "wpDR": q8(_pair_m(w_proj, KT)),
        "w1g": np.ascontiguousarray(
            w1s.reshape(MT1 // 4, 4, 128, C).transpose(0, 2, 1, 3)
            .reshape(MT1 // 4, 128, 4 * C)).astype(bf),
        "w2DR": q8(_pair_m(w_fc2, KT)),
        "sel2in": sel2,
        # qkv biases ride the fp8-scaled psum; proj/fc2 biases likewise
        "b_all": np.concatenate(
            [bqkv_e * WSCALE, b_proj * (OSC * WSCALE), bfc1_e,
             b_fc2 * WSCALE])[None, :].astype(bf),
    }
    has_bias = {
        "qk": bool(np.any(bqkv_e[:2 * C])),
        "v": bool(np.any(bqkv_e[2 * C:])),
        "proj": bool(np.any(b_proj)),
        "fc1": bool(np.any(bfc1_e)),
        "fc2": bool(np.any(b_fc2)),
    }

    in_maps = []
    for c in range(8):
        b, half = c // 2, c % 2
        xb = x[b]
        if half:
            xb = np.concatenate([xb[NQ:], xb[:NQ]], axis=0)
        xt = np.ascontiguousarray(xb.T)
        m = {"xTp": _feat_pack(xt[:, :NQ], NQ),
             "xbp": _feat_pack(xt, NTOK).astype(bf),
             **shared}
        in_maps.append(m)
    return in_maps, has_bias


def kernel(**inputs):
    in_maps, has_bias = _prep(inputs)
    key = tuple(sorted(has_bias.items()))
    if key not in _cache:
        nc = build_program(has_bias)
        _split_wide_waits(nc, 1)
        _cache[key] = nc
    nc = _cache[key]

    res = bass_utils.run_bass_kernel_spmd(
        nc, in_maps, core_ids=list(range(8)), trace=False)

    x = np.asarray(inputs["x"])
    out = np.empty((4, NTOK, C), dtype=np.float32)
    for c in range(8):
        b, half = c // 2, c % 2
        out[b, half * NQ:(half + 1) * NQ, :] = res.results[c]["yT"].T
    return out.astype(x.dtype, copy=False)
